# revision 33
# baseline (speedup 1.0000x reference)
"""EGNN (N=384, D=3, H=128, L=4) Bass kernel for 8 TRN2 NeuronCores. v3.

Sharding: rows of the N x N edge grid split across 8 cores (48 rows each).
Per layer: coord MLP first (phi -> x update -> x AllGather issued mid-layer),
then edge MLP; gated message row-sums (msum) overlap the remaining edge
groups via 12-row attention sub-accumulators. The node MLP runs on the LOCAL
48 nodes only and the new h rows are AllGathered in fp16.

Edge-grid matmuls run in fp16 (fp32 PSUM accumulation) - fp16 keeps weight
quantization ~1e-3 where bf16 was ~1e-2. MLP groups take rows (g, g+16,
g+32) so the three K=2 distance matmuls of a group occupy PE row-blocks
0/32/64 and execute concurrently (tile-position row packing). The gated
message multiply uses a gpsimd partition_broadcast of the gate rows instead
of a PE broadcast matmul. Sigmoid is expressed via tanh to stay in the silu
ACT table set.

Self-contained: shapes hardcoded, inputs are the full unsharded arrays.
"""
import numpy as np

N, D, H, L = 384, 3, 128, 4
NC = 8
NI = N // NC          # 48 rows per core
NJ = N                # 384 cols
G = 3                 # i-rows per MLP group (rows g, g+16, g+32)
NGRP = 16             # groups per pass
SUB = 12              # attention sub-accumulator rows (4 groups each)
NSUB = 4
EMB_ROWS = N * H // NC  # 6144 emb_w rows per core

_cache = {}


def _row(g, r):
    return g + 16 * r


def _slot_to_row(k, s):
    # slot s of sub k -> actual row index
    return (4 * k + s % 4) + 16 * (s // 4)


def _build_nc():
    import concourse.bass as bass
    import concourse.bacc as bacc
    import concourse.tile as tile
    from concourse import mybir

    F32 = mybir.dt.float32
    FP16 = mybir.dt.float16
    AF = mybir.ActivationFunctionType
    OP = mybir.AluOpType

    nc = bacc.Bacc(None, target_bir_lowering=False)

    def P(name, shape, dt=F32):
        return nc.declare_dram_parameter(name, list(shape), dt, isOutput=False)

    # per-core inputs
    embw = P("embw", (EMB_ROWS, NJ))
    embbT = P("embbT", (H, NI))
    x0my = P("x0my", (NI, D))
    maskc = P("maskc", (NI, NJ))
    eyec = P("eyec", (NI, NJ))
    hm4 = P("hm4", (SUB, NSUB * NJ))          # 0.5*mask rows, slot layout
    # shared inputs (fp16 weights for the edge-grid MLP passes)
    x0rows = P("x0rows", (1, D * NJ))
    c1hiT = P("c1hiT", (L, H, H))             # f32 (A matmul)
    c1hjT = P("c1hjT", (L, H, H), FP16)
    c1drep = P("c1drep", (L, 16 * H), FP16)
    cb1 = P("cb1", (H, L))
    c2T = P("c2T", (L, H, H), FP16)
    cb2 = P("cb2", (H, L))
    c3w = P("c3w", (L, H, 2 * NI - 1), FP16)  # phi window (95 wide)
    cb3c = P("cb3c", (NI, L))
    e1hiT = P("e1hiT", (L - 1, H, H))         # f32 (A matmul)
    e1hjT = P("e1hjT", (L - 1, H, H), FP16)
    e1drep = P("e1drep", (L - 1, 16 * H), FP16)
    ones6k = P("ones6k", (1, 16 * NJ), FP16)
    eb1 = P("eb1", (H, L - 1))
    e2T = P("e2T", (L - 1, H, H), FP16)
    eb2 = P("eb2", (H, L - 1))
    attw = P("attw", (L - 1, H, 2 * SUB - 1), FP16)  # att window (23 wide)
    nw1hT = P("nw1hT", (L - 1, H, H))
    nw1mT = P("nw1mT", (L - 1, H, H))
    nb1 = P("nb1", (H, L - 1))
    nw2T = P("nw2T", (L - 1, H, H))
    nb2 = P("nb2", (H, L - 1))
    ones128 = P("ones128", (1, H), FP16)

    o_x = nc.declare_dram_parameter("o_x", [N, D], F32, isOutput=True)

    # collective bounce buffers (h in fp16; x in f32)
    hag_in = [nc.dram_tensor(f"hag_in{l}", [H, NI], FP16) for l in range(L)]
    hag_out = [nc.dram_tensor(f"hag_out{l}", [NC * H, NI], FP16,
                              addr_space="Shared") for l in range(L)]
    hag_h = [[nc.dram_tensor(f"hag_h{l}_{h}", [H, 24], FP16)
              for h in range(2)] for l in range(L)]
    hag_ho = [[nc.dram_tensor(f"hag_ho{l}_{h}", [NC * H, 24], FP16,
                              addr_space="Shared") for h in range(2)]
              for l in range(L)]
    xag_in = [nc.dram_tensor(f"xag_in{l}", [D, NI], F32) for l in range(L - 1)]
    xag_in.append(nc.dram_tensor(f"xag_in3", [NI, D], F32))
    xag_out = [nc.dram_tensor(f"xag_out{l}", [NC, D, NI], F32, addr_space="Shared")
               for l in range(L - 1)]
    xag_out.append(nc.dram_tensor(f"xag_out3", [N, D], F32, addr_space="Shared"))
    rg = [list(range(NC))]
    gmd = [nc.dram_tensor(f"gmd{l}", [1, NSUB * SUB * NJ], FP16)
           for l in range(L - 1)]
    wag_in = nc.dram_tensor("wag_in", [1, 8], FP16)
    wag_out = nc.dram_tensor("wag_out", [NC, 8], FP16, addr_space="Shared")
    hag_q = [nc.dram_tensor(f"hag_q{q}", [H, 12], FP16) for q in range(4)]
    hag_qo = [nc.dram_tensor(f"hag_qo{q}", [NC * H, 12], FP16,
                             addr_space="Shared") for q in range(4)]

    with tile.TileContext(nc) as tc:
        with (
            tc.tile_pool(name="consts", bufs=1) as consts,
            tc.tile_pool(name="embp", bufs=2) as embp,
            tc.tile_pool(name="work", bufs=2) as work,
            tc.tile_pool(name="slab", bufs=1) as slabp,
            tc.tile_pool(name="cp", bufs=2) as cp,
            tc.tile_pool(name="mgp", bufs=2) as mgp,
            tc.tile_pool(name="ps_mlp", bufs=2, space="PSUM") as ps_mlp,
            tc.tile_pool(name="ps_acc", bufs=2, space="PSUM") as ps_acc,
        ):
            # warm up the collective path so the first real AllGather
            # doesn't pay the cold-start cost
            wz = cp.tile([1, 8], FP16, tag="wz")
            nc.vector.memset(wz, 0.0)
            nc.sync.dma_start(out=wag_in[:], in_=wz)
            nc.gpsimd.collective_compute(
                "AllGather", OP.bypass, replica_groups=rg,
                ins=[wag_in[:]], outs=[wag_out[:]],
            )

            # ---- embedding row-sum first: its DMAs dominate the
            # startup critical path, so issue them before the const loads
            embbT_sb = consts.tile([H, NI], F32, tag="embbT")
            nc.scalar.dma_start(out=embbT_sb, in_=embbT[:])

            h_my = cp.tile([H, NI], F32, tag="hmy")

            EB = 2  # nodes per reduce; each node block is one dense DMA
            for t in range(NI // EB):
                et = embp.tile([H, EB, NJ], F32, tag="embt")
                for a in range(EB):
                    n = EB * t + a
                    eng = nc.sync if n % 2 == 0 else nc.scalar
                    eng.dma_start(out=et[:, a:a + 1, :],
                                  in_=embw[n * H:(n + 1) * H, :])
                nc.vector.tensor_reduce(
                    out=h_my[:, t * EB:(t + 1) * EB], in_=et,
                    axis=mybir.AxisListType.X, op=OP.add,
                )

            # ---- load constants (cycled over queues) ----
            _ldq = [0]

            def load(pname, ap_in, shape, dt=F32):
                t = consts.tile(list(shape), dt, tag=pname)
                eng = [nc.scalar, nc.sync][_ldq[0] % 2]
                _ldq[0] += 1
                eng.dma_start(out=t, in_=ap_in)
                return t

            c1hiT_sb = load("c1hiT", c1hiT.rearrange("l p x -> p l x"), (H, L, H))
            c1hjT_sb = load("c1hjT", c1hjT.rearrange("l p x -> p l x"), (H, L, H),
                            FP16)
            c2T_sb = load("c2T", c2T.rearrange("l p x -> p l x"), (H, L, H), FP16)
            c3w_sb = load("c3w", c3w.rearrange("l p x -> p l x"),
                          (H, L, 2 * NI - 1), FP16)
            cb1_sb = load("cb1", cb1[:], (H, L))
            cb2_sb = load("cb2", cb2[:], (H, L))
            cb3c_sb = load("cb3c", cb3c[:], (NI, L))
            e1hiT_sb = load("e1hiT", e1hiT.rearrange("l p x -> p l x"),
                            (H, L - 1, H))
            e1hjT_sb = load("e1hjT", e1hjT.rearrange("l p x -> p l x"),
                            (H, L - 1, H), FP16)
            e2T_sb = load("e2T", e2T.rearrange("l p x -> p l x"), (H, L - 1, H),
                          FP16)
            attw_sb = load("attw", attw.rearrange("l p x -> p l x"),
                           (H, L - 1, 2 * SUB - 1), FP16)
            eb1_sb = load("eb1", eb1[:], (H, L - 1))
            eb2_sb = load("eb2", eb2[:], (H, L - 1))
            nw1hT_sb = load("nw1hT", nw1hT.rearrange("l p x -> p l x"),
                            (H, L - 1, H))
            nw1mT_sb = load("nw1mT", nw1mT.rearrange("l p x -> p l x"),
                            (H, L - 1, H))
            nw2T_sb = load("nw2T", nw2T.rearrange("l p x -> p l x"),
                           (H, L - 1, H))
            nb1_sb = load("nb1", nb1[:], (H, L - 1))
            nb2_sb = load("nb2", nb2[:], (H, L - 1))
            maskc_sb = load("maskc", maskc[:], (NI, NJ))
            eyec_sb = load("eyec", eyec[:], (NI, NJ))
            hm4_sb = load("hm4", hm4[:], (SUB, NSUB * NJ))
            x0my_sb = load("x0my", x0my[:], (NI, D))
            c1drep_sb = load("c1drep", c1drep[:], (L, 16 * H), FP16)
            ones_sb = load("ones128", ones128[:], (1, H), FP16)
            e1drep_sb = load("e1drep", e1drep[:], (L - 1, 16 * H), FP16)

            # d2ones ping-pong (row 32k: d2 strip k; row 32k+1: ones)
            d2o = [consts.tile([66, 16 * NJ], FP16, tag=f"d2o{k}",
                               name=f"d2o{k}")
                   for k in range(2)]
            for t in d2o:
                for k in range(3):
                    nc.sync.dma_start(out=t[32 * k + 1:32 * k + 2, :],
                                      in_=ones6k[:])
            combC = consts.tile([66, 16 * H], FP16, tag="combC")
            combE = consts.tile([66, 16 * H], FP16, tag="combE")

            # ---- phase 0 tail: bias add + single initial h-AG ----
            nc.vector.tensor_tensor(out=h_my[:], in0=h_my, in1=embbT_sb,
                                    op=OP.add)
            hb0 = cp.tile([H, NI], FP16, tag="hagb0")
            nc.vector.tensor_copy(hb0, h_my)
            nc.sync.dma_start(out=hag_in[0][:], in_=hb0)
            nc.gpsimd.collective_compute(
                "AllGather", OP.bypass, replica_groups=rg,
                ins=[hag_in[0][:]], outs=[hag_out[0][:]],
            )

            def send_h(l, hmy_t):
                hb = cp.tile([H, NI], FP16, tag="hagb")
                nc.vector.tensor_copy(hb, hmy_t)
                nc.sync.dma_start(out=hag_in[l][:], in_=hb)
                nc.gpsimd.collective_compute(
                    "AllGather", OP.bypass, replica_groups=rg,
                    ins=[hag_in[l][:]], outs=[hag_out[l][:]],
                )

            def recv_h(l):
                hT = cp.tile([H, NJ], FP16, tag="hT")
                engs = [nc.sync, nc.gpsimd, nc.scalar]
                for r in range(NC):
                    engs[r % 3].dma_start(out=hT[:, r * NI:(r + 1) * NI],
                                          in_=hag_out[l][r * H:(r + 1) * H, :])
                return hT

            def send_h_half(l, half, hmy_t):
                # columns {0-7,16-23,32-39} (half 0) / {8-15,24-31,40-47}
                hv = hmy_t.rearrange("p (b c) -> p b c", b=3)
                hb = cp.tile([H, 3, 8], FP16, tag=f"hagb{half}")
                nc.vector.tensor_copy(hb, hv[:, :, 8 * half:8 * half + 8])
                nc.sync.dma_start(out=hag_h[l][half][:], in_=hb)
                nc.gpsimd.collective_compute(
                    "AllGather", OP.bypass, replica_groups=rg,
                    ins=[hag_h[l][half][:]], outs=[hag_ho[l][half][:]],
                )

            def recv_h_halves(l):
                hT = cp.tile([H, NJ], FP16, tag="hT")
                hv = hT.rearrange("p (rr b c) -> p rr b c", rr=NC, b=3)
                engs = [nc.sync, nc.gpsimd, nc.scalar]
                for half in range(2):
                    for r in range(NC):
                        src_ap = hag_ho[l][half][r * H:(r + 1) * H, :]
                        engs[r % 3].dma_start(
                            out=hv[:, r, :, 8 * half:8 * half + 8],
                            in_=src_ap.rearrange("p (b c) -> p b c", b=3))
                return hT

            x_my = x0my_sb

            # ---- per-layer prep: diff/d2 + fp16 d2 strips ----
            def prep_geom(l, x_t):
                diff = []
                for c in range(D):
                    xb = cp.tile([NI, NJ], F32, tag=f"xb{c}")
                    if l == 0:
                        bsrc = bass.AP(tensor=x0rows, offset=c * NJ,
                                       ap=[[0, NI], [1, NJ]])
                    else:
                        bsrc = bass.AP(tensor=xag_out[l - 1], offset=c * NI,
                                       ap=[[0, NI], [D * NI, NC], [1, NI]])
                    nc.sync.dma_start(out=xb, in_=bsrc)
                    dc = cp.tile([NI, NJ], F32, tag=f"diff{c}")
                    nc.vector.tensor_scalar(
                        out=dc, in0=xb, scalar1=x_t[:, c:c + 1], scalar2=None,
                        op0=OP.subtract,
                    )
                    diff.append(dc)
                d2 = cp.tile([NI, NJ], F32, tag="d2")
                tmp = cp.tile([NI, NJ], F32, tag="ctmp")
                nc.vector.tensor_tensor(out=d2, in0=diff[0], in1=diff[0],
                                        op=OP.mult)
                nc.vector.tensor_tensor(out=tmp, in0=diff[1], in1=diff[1],
                                        op=OP.mult)
                nc.vector.tensor_tensor(out=d2, in0=d2, in1=tmp, op=OP.add)
                nc.vector.tensor_tensor(out=tmp, in0=diff[2], in1=diff[2],
                                        op=OP.mult)
                nc.vector.tensor_tensor(out=d2, in0=d2, in1=tmp, op=OP.add)
                d2s = cp.tile([NI, NJ], F32, tag="d2s")
                nc.vector.tensor_tensor(out=d2s, in0=d2, in1=eyec_sb, op=OP.add)
                d2b = cp.tile([NI, NJ], FP16, tag="d2b")
                nc.vector.tensor_copy(d2b, d2)
                dst = d2o[l % 2]
                for k in range(3):
                    nc.sync.dma_start(out=dst[32 * k:32 * k + 1, :],
                                      in_=d2b[16 * k:16 * (k + 1), :])
                # u = 1/(1+sqrt(d2s)) off the layer-end critical path
                dn = cp.tile([NI, NJ], F32, tag="dn")
                nc.scalar.activation(out=dn, in_=d2s, func=AF.Sqrt)
                nc.vector.tensor_scalar(out=dn, in0=dn, scalar1=1.0,
                                        scalar2=None, op0=OP.add)
                u = cp.tile([NI, NJ], F32, tag="u")
                nc.vector.reciprocal(out=u, in_=dn)
                return diff, u

            def make_comb(l, hiT_l, wdrep_l, comb, hmy_t, atag):
                At_ps = ps_acc.tile([NI, H], F32, tag="acc")
                nc.tensor.matmul(At_ps, lhsT=hmy_t, rhs=hiT_l, start=True,
                                 stop=True)
                At = cp.tile([NI, H], FP16, tag=atag)
                nc.vector.tensor_copy(At, At_ps)
                for k in range(3):
                    nc.sync.dma_start(out=comb[32 * k:32 * k + 1, :],
                                      in_=wdrep_l)
                    nc.sync.dma_start(out=comb[32 * k + 1:32 * k + 2, :],
                                      in_=At[16 * k:16 * (k + 1), :])

            diff, u_t = prep_geom(0, x0my_sb)
            hT_bf = recv_h(0)
            make_comb(0, c1hiT_sb[:, 0, :], c1drep_sb[0:1, :], combC, h_my,
                      "AtC")
            make_comb(0, e1hiT_sb[:, 0, :], e1drep_sb[0:1, :], combE, h_my,
                      "AtE")

            for l in range(L):
                last = l == L - 1
                d2l = d2o[l % 2]
                hT_l = hT_bf
                hmy_l = h_my

                # one MLP group g covers rows (g, g+16, g+32): the three K=2
                # distance matmuls hit PE row-blocks 0/32/64 and run packed.
                def emit_group(comb, b1c, hjT, w2T, b2c, g, slab):
                    pre = ps_mlp.tile([H, G * 512], F32, tag="mlp")
                    for r in range(G):
                        nc.tensor.matmul(
                            pre[:, r * 512:r * 512 + NJ],
                            lhsT=comb[32 * r:32 * r + 2,
                                      g * H:(g + 1) * H],
                            rhs=d2l[32 * r:32 * r + 2,
                                    g * NJ:(g + 1) * NJ],
                            start=True, stop=False)
                    for r in range(G):
                        nc.tensor.matmul(
                            pre[:, r * 512:r * 512 + NJ],
                            lhsT=hjT, rhs=hT_l,
                            start=False, stop=True)
                    t1 = work.tile([H, G * NJ], FP16, tag="t1", bufs=3)
                    nc.scalar.activation(
                        out=t1[:, :].rearrange("p (r c) -> p r c", r=G),
                        in_=pre.rearrange("p (r c) -> p r c", r=G)[:, :, 0:NJ],
                        func=AF.Silu, bias=b1c, scale=1.0,
                    )
                    z2 = pre  # reuse the pre banks: t1 has been read out
                    nc.tensor.matmul(z2[:, 0:512], lhsT=w2T, rhs=t1[:, 0:512],
                                     start=True, stop=True)
                    nc.tensor.matmul(z2[:, 512:1024], lhsT=w2T,
                                     rhs=t1[:, 512:1024], start=True, stop=True)
                    nc.tensor.matmul(z2[:, 1024:1152], lhsT=w2T,
                                     rhs=t1[:, 1024:1152], start=True,
                                     stop=True)
                    if slab is not None:
                        t2 = slab[:, g * (G * NJ):(g + 1) * (G * NJ)]
                    else:
                        t2 = work.tile([H, G * NJ], FP16, tag="t2", bufs=3)
                    nc.scalar.activation(
                        out=t2, in_=z2[:, 0:G * NJ],
                        func=AF.Silu, bias=b2c, scale=1.0,
                    )
                    return t2

                phi_ps = ps_acc.tile([H, NJ], F32, tag="acc")
                pendC = []

                def flush_coord(item):
                    t2p, gp = item
                    for r in range(G):
                        i = _row(gp, r)
                        nc.tensor.matmul(
                            phi_ps[0:NI, :],
                            lhsT=c3w_sb[:, l, (NI - 1) - i:(2 * NI - 1) - i],
                            rhs=t2p[:, r * NJ:(r + 1) * NJ],
                            start=(gp == 0 and r == 0),
                            stop=(gp == NGRP - 1 and r == G - 1),
                        )

                xnew = cp.tile([NI, D], F32, tag="xnew")

                def x_chain():
                    u = u_t
                    phis = cp.tile([NI, NJ], F32, tag="phis")
                    nc.vector.tensor_scalar(out=phis, in0=phi_ps[0:NI, :],
                                            scalar1=cb3c_sb[:, l:l + 1],
                                            scalar2=None, op0=OP.add)
                    s = cp.tile([NI, NJ], F32, tag="s")
                    nc.vector.tensor_tensor(out=s, in0=phis, in1=u, op=OP.mult)
                    nc.vector.tensor_tensor(out=s, in0=s, in1=maskc_sb,
                                            op=OP.mult)
                    for c in range(D):
                        xm = mgp.tile([NI, NJ], F32, tag="xm")
                        xcol = cp.tile([NI, 1], F32, tag=f"xcol{c}")
                        nc.vector.scalar_tensor_tensor(
                            out=xm, in0=diff[c], scalar=1.0, in1=s,
                            op0=OP.mult, op1=OP.mult, accum_out=xcol)
                        nc.vector.tensor_tensor(out=xnew[:, c:c + 1], in0=xcol,
                                                in1=x_my[:, c:c + 1],
                                                op=OP.add)
                    if not last:
                        nc.sync.dma_start(out=xag_in[l]
                                          .rearrange("c n -> n c"), in_=xnew)
                    else:
                        nc.sync.dma_start(out=xag_in[l][:], in_=xnew)
                    nc.gpsimd.collective_compute(
                        "AllGather", OP.bypass, replica_groups=rg,
                        ins=[xag_in[l][:]], outs=[xag_out[l][:]],
                    )

                if last:
                    for g in range(NGRP):
                        t2c = emit_group(combC, cb1_sb[:, l:l + 1],
                                         c1hjT_sb[:, l, :], c2T_sb[:, l, :],
                                         cb2_sb[:, l:l + 1], g, None)
                        pendC.append((t2c, g))
                        if len(pendC) > 1:
                            flush_coord(pendC.pop(0))
                    while pendC:
                        flush_coord(pendC.pop(0))
                    x_chain()
                    nc.sync.dma_start(out=o_x[:], in_=xag_out[l][:])
                    break

                # ---------- merged coord+edge phase ----------
                m2slab = slabp.tile([H, NI * NJ], FP16, tag="m2")
                msumT = cp.tile([H, NI], F32, tag="msumT")
                att_sub = [None] * NSUB
                gmb = [None] * NSUB
                pendE = []
                subs_ready = [0]
                subs_done = [0]

                def sigma_sub(k):
                    tt = cp.tile([SUB, NJ], F32, tag="tt")
                    nc.scalar.activation(out=tt, in_=att_sub[k][0:SUB, :],
                                         func=AF.Tanh, scale=0.5)
                    gm = cp.tile([SUB, NJ], FP16, tag="gmb")
                    nc.vector.scalar_tensor_tensor(
                        out=gm, in0=tt, scalar=1.0,
                        in1=hm4_sb[:, k * NJ:(k + 1) * NJ],
                        op0=OP.add, op1=OP.mult)
                    gmb[k] = gm
                    nc.scalar.dma_start(
                        out=gmd[l][0:1, k * SUB * NJ:(k + 1) * SUB * NJ],
                        in_=gm)
                    subs_ready[0] += 1

                def flush_edge(gp):
                    t2p = m2slab[:, gp * (G * NJ):(gp + 1) * (G * NJ)]
                    k = gp // 4
                    for r in range(G):
                        slot = (gp % 4) + 4 * r
                        if gp % 4 == 0 and r == 0:
                            att_sub[k] = ps_acc.tile([SUB, NJ], F32,
                                                     tag="acc",
                                                     name=f"attsub{k}")
                        nc.tensor.matmul(
                            att_sub[k][:, :],
                            lhsT=attw_sb[:, l, (SUB - 1) - slot:
                                         (2 * SUB - 1) - slot],
                            rhs=t2p[:, r * NJ:(r + 1) * NJ],
                            start=(gp % 4 == 0 and r == 0),
                            stop=(gp % 4 == 3 and r == G - 1),
                        )
                    if gp % 4 == 3:
                        sigma_sub(k)

                def msum_sub(k):
                    # gate rows were staged in DRAM at sigma time; read each
                    # back with a stride-0 partition-broadcast DMA (cheap,
                    # dependency-free by now) and multiply-reduce on DVE
                    dengs = [nc.sync, nc.gpsimd, nc.scalar]
                    for s in range(SUB):
                        i = _slot_to_row(k, s)
                        off = (i % 16) * (G * NJ) + (i // 16) * NJ
                        gbb = mgp.tile([H, NJ], FP16, tag="gbb", bufs=6,
                                       name="gbb")
                        dengs[s % 3].dma_start(
                            out=gbb,
                            in_=bass.AP(tensor=gmd[l],
                                        offset=(k * SUB + s) * NJ,
                                        ap=[[0, H], [1, NJ]]))
                        mg = mgp.tile([H, NJ], FP16, tag="mg")
                        nc.vector.scalar_tensor_tensor(
                            out=mg, in0=m2slab[:, off:off + NJ],
                            scalar=1.0, in1=gbb,
                            op0=OP.mult, op1=OP.mult,
                            accum_out=msumT[:, i:i + 1])

                h_my_new = cp.tile([H, NI], F32, tag="hmy")

                def node_half(half):
                    # node MLP on 24 local nodes: cols {0-7,16-23,32-39}+8*half
                    sl = slice(8 * half, 8 * half + 8)
                    hv = hmy_l.rearrange("p (b c) -> p b c", b=3)
                    mv = msumT.rearrange("p (b c) -> p b c", b=3)
                    z1m = ps_acc.tile([H, 24], F32, tag="acc",
                                      name=f"z1m{half}")
                    nc.tensor.matmul(z1m, lhsT=nw1hT_sb[:, l, :],
                                     rhs=hv[:, :, sl], start=True, stop=False)
                    nc.tensor.matmul(z1m, lhsT=nw1mT_sb[:, l, :],
                                     rhs=mv[:, :, sl], start=False, stop=True)
                    t1nm = cp.tile([H, 24], F32, tag=f"t1nm{half}")
                    nc.scalar.activation(out=t1nm, in_=z1m, func=AF.Silu,
                                         bias=nb1_sb[:, l:l + 1], scale=1.0)
                    z2m = ps_acc.tile([H, 24], F32, tag="acc",
                                      name=f"z2m{half}")
                    nc.tensor.matmul(z2m, lhsT=nw2T_sb[:, l, :], rhs=t1nm,
                                     start=True, stop=True)
                    nv = h_my_new.rearrange("p (b c) -> p b c", b=3)
                    nc.vector.tensor_scalar(out=nv[:, :, sl], in0=z2m,
                                            scalar1=nb2_sb[:, l:l + 1],
                                            scalar2=None, op0=OP.add)
                    send_h_half(l + 1, half, h_my_new)

                # slot schedule: 2 coord : 1 edge until coords exhausted,
                # then the remaining edge groups
                slots = []
                for q in range(8):
                    slots += ["C", "C", "E"]
                slots += ["E"] * 8
                ci = [0]
                ei = [0]
                xdone = [False]
                prep_at = [None]

                for si, kind in enumerate(slots):
                    if kind == "C":
                        t2c = emit_group(combC, cb1_sb[:, l:l + 1],
                                         c1hjT_sb[:, l, :], c2T_sb[:, l, :],
                                         cb2_sb[:, l:l + 1], ci[0], None)
                        pendC.append((t2c, ci[0]))
                        ci[0] += 1
                        if len(pendC) > 1:
                            flush_coord(pendC.pop(0))
                    else:
                        emit_group(combE, eb1_sb[:, l:l + 1],
                                   e1hjT_sb[:, l, :], e2T_sb[:, l, :],
                                   eb2_sb[:, l:l + 1], ei[0], m2slab)
                        pendE.append(ei[0])
                        ei[0] += 1
                        if len(pendE) > 1:
                            flush_edge(pendE.pop(0))
                    if ci[0] == NGRP and not xdone[0]:
                        while pendC:
                            flush_coord(pendC.pop(0))
                        x_chain()
                        xdone[0] = True
                        prep_at[0] = si + 3
                    if prep_at[0] is not None and si == prep_at[0]:
                        diff, u_t = prep_geom(l + 1, xnew)
                    while xdone[0] and subs_done[0] < subs_ready[0]:
                        msum_sub(subs_done[0])
                        subs_done[0] += 1
                        if subs_done[0] == 2:
                            node_half(0)
                while pendE:
                    flush_edge(pendE.pop(0))
                while subs_done[0] < subs_ready[0]:
                    msum_sub(subs_done[0])
                    subs_done[0] += 1
                    if subs_done[0] == 2:
                        node_half(0)
                node_half(1)
                h_my = h_my_new

                # prep for next layer while the h AllGather flies
                make_comb(l + 1, c1hiT_sb[:, l + 1, :],
                          c1drep_sb[l + 1:l + 2, :], combC, h_my, "AtC")
                if l + 1 < L - 1:
                    make_comb(l + 1, e1hiT_sb[:, l + 1, :],
                              e1drep_sb[l + 1:l + 2, :], combE, h_my, "AtE")
                x_my = xnew
                hT_bf = recv_h_halves(l + 1)

    nc.finalize()
    return nc


def _prep_inputs(inputs):
    """Host-side prep: per-core input maps from full arrays."""
    F16 = np.float16
    f = lambda a: np.ascontiguousarray(np.asarray(a), dtype=np.float32)
    b = lambda a: np.ascontiguousarray(np.asarray(a, dtype=np.float32)
                                       .astype(F16))
    x_inp = f(inputs["x_inp"])
    emb_w = f(inputs["emb_w"])
    emb_b = f(inputs["emb_b"])
    coord_w1 = f(inputs["coord_w1"])
    coord_b1 = f(inputs["coord_b1"])
    coord_w2 = f(inputs["coord_w2"])
    coord_b2 = f(inputs["coord_b2"])
    coord_w3 = f(inputs["coord_w3"])
    coord_b3 = f(inputs["coord_b3"])
    edge_w1 = f(inputs["edge_w1"])
    edge_b1 = f(inputs["edge_b1"])
    edge_w2 = f(inputs["edge_w2"])
    edge_b2 = f(inputs["edge_b2"])
    node_w1 = f(inputs["node_w1"])
    node_b1 = f(inputs["node_b1"])
    node_w2 = f(inputs["node_w2"])
    node_b2 = f(inputs["node_b2"])
    att_w = f(inputs["att_w"])

    x0 = x_inp.reshape(N, D)
    eye = np.eye(N, dtype=np.float32)

    def stackT(w, lo, hi):
        return np.ascontiguousarray(
            np.stack([w[l, :, lo:hi].T for l in range(w.shape[0])]))

    def win(w3, S):
        nl = w3.shape[0]
        out = np.zeros((nl, H, 2 * S - 1), np.float32)
        out[:, :, S - 1] = w3[:, 0, :]
        return out

    shared = dict(
        x0rows=np.ascontiguousarray(x0.T.reshape(1, D * N)),
        c1hiT=stackT(coord_w1, 0, H),
        c1hjT=b(stackT(coord_w1, H, 2 * H)),
        c1drep=b(np.tile(coord_w1[:, :, 2 * H], (1, 16))),
        cb1=np.ascontiguousarray(coord_b1.T),
        c2T=b(np.stack([coord_w2[l].T for l in range(L)])),
        cb2=np.ascontiguousarray(coord_b2.T),
        c3w=b(win(coord_w3, NI)),
        cb3c=np.ascontiguousarray(
            np.broadcast_to(coord_b3[:, 0][None, :], (NI, L))),
        e1hiT=stackT(edge_w1, 0, H),
        e1hjT=b(stackT(edge_w1, H, 2 * H)),
        e1drep=b(np.tile(edge_w1[:, :, 2 * H], (1, 16))),
        ones6k=b(np.ones((1, 16 * NJ), np.float32)),
        eb1=np.ascontiguousarray(edge_b1.T),
        e2T=b(np.stack([edge_w2[l].T for l in range(L - 1)])),
        eb2=np.ascontiguousarray(edge_b2.T),
        attw=b(win(att_w, SUB)),
        nw1hT=stackT(node_w1, 0, H),
        nw1mT=stackT(node_w1, H, 2 * H),
        nb1=np.ascontiguousarray(node_b1.T),
        nw2T=np.ascontiguousarray(np.stack([node_w2[l].T
                                            for l in range(L - 1)])),
        nb2=np.ascontiguousarray(node_b2.T),
        ones128=b(np.ones((1, H), np.float32)),
    )
    in_maps = []
    for c in range(NC):
        m = dict(shared)
        m["embw"] = np.ascontiguousarray(
            emb_w[c * EMB_ROWS:(c + 1) * EMB_ROWS, :])
        m["embbT"] = np.ascontiguousarray(
            emb_b[c * EMB_ROWS:(c + 1) * EMB_ROWS].reshape(NI, H).T)
        m["x0my"] = np.ascontiguousarray(x0[c * NI:(c + 1) * NI, :])
        mk = 1.0 - eye[c * NI:(c + 1) * NI, :]
        m["maskc"] = np.ascontiguousarray(mk)
        m["eyec"] = np.ascontiguousarray(eye[c * NI:(c + 1) * NI, :])
        # 0.5*mask rows in (sub, slot) layout
        hm = np.zeros((SUB, NSUB * NJ), np.float32)
        for k in range(NSUB):
            for s in range(SUB):
                hm[s, k * NJ:(k + 1) * NJ] = 0.5 * mk[_slot_to_row(k, s), :]
        m["hm4"] = np.ascontiguousarray(hm)
        in_maps.append(m)
    return in_maps


def _run(inputs, trace=False, **kw):
    from concourse.bass_utils import run_bass_kernel_spmd
    if "nc" not in _cache:
        _cache["nc"] = _build_nc()
    in_maps = _prep_inputs(inputs)
    return run_bass_kernel_spmd(_cache["nc"], in_maps, list(range(NC)),
                                trace=trace, **kw)


def kernel(**inputs) -> np.ndarray:
    res = _run(inputs)
    return np.asarray(res.results[0]["o_x"], dtype=np.float32).reshape(N * D)


# revision 34
# speedup vs baseline: 1.0499x; 1.0499x over previous
"""EGNN (N=384, D=3, H=128, L=4) Bass kernel for 8 TRN2 NeuronCores. v3.

Sharding: rows of the N x N edge grid split across 8 cores (48 rows each).
Per layer: coord MLP first (phi -> x update -> x AllGather issued mid-layer),
then edge MLP; gated message row-sums (msum) overlap the remaining edge
groups via 12-row attention sub-accumulators. The node MLP runs on the LOCAL
48 nodes only and the new h rows are AllGathered in fp16.

Edge-grid matmuls run in fp16 (fp32 PSUM accumulation) - fp16 keeps weight
quantization ~1e-3 where bf16 was ~1e-2. MLP groups take rows (g, g+16,
g+32) so the three K=2 distance matmuls of a group occupy PE row-blocks
0/32/64 and execute concurrently (tile-position row packing). The gated
message multiply uses a gpsimd partition_broadcast of the gate rows instead
of a PE broadcast matmul. Sigmoid is expressed via tanh to stay in the silu
ACT table set.

Self-contained: shapes hardcoded, inputs are the full unsharded arrays.
"""
import numpy as np

N, D, H, L = 384, 3, 128, 4
NC = 8
NI = N // NC          # 48 rows per core
NJ = N                # 384 cols
G = 3                 # i-rows per MLP group (rows g, g+16, g+32)
NGRP = 16             # groups per pass
SUB = 12              # attention sub-accumulator rows (4 groups each)
NSUB = 4
EMB_ROWS = N * H // NC  # 6144 emb_w rows per core

_cache = {}


def _row(g, r):
    return g + 16 * r


def _slot_to_row(k, s):
    # slot s of sub k -> actual row index
    return (4 * k + s % 4) + 16 * (s // 4)


def _build_nc():
    import concourse.bass as bass
    import concourse.bacc as bacc
    import concourse.tile as tile
    from concourse import mybir

    F32 = mybir.dt.float32
    FP16 = mybir.dt.float16
    AF = mybir.ActivationFunctionType
    OP = mybir.AluOpType

    nc = bacc.Bacc(None, target_bir_lowering=False)

    def P(name, shape, dt=F32):
        return nc.declare_dram_parameter(name, list(shape), dt, isOutput=False)

    # per-core inputs
    embw = P("embw", (EMB_ROWS, NJ))
    embbT = P("embbT", (H, NI))
    x0my = P("x0my", (NI, D))
    maskc = P("maskc", (NI, NJ))
    eyec = P("eyec", (NI, NJ))
    hm4 = P("hm4", (SUB, NSUB * NJ))          # 0.5*mask rows, slot layout
    # shared inputs (fp16 weights for the edge-grid MLP passes)
    x0rows = P("x0rows", (1, D * NJ))
    c1hiT = P("c1hiT", (L, H, H))             # f32 (A matmul)
    c1hjT = P("c1hjT", (L, H, H), FP16)
    c1drep = P("c1drep", (L, 16 * H), FP16)
    cb1 = P("cb1", (H, L))
    c2T = P("c2T", (L, H, H), FP16)
    cb2 = P("cb2", (H, L))
    c3w = P("c3w", (L, H, 2 * NI - 1), FP16)  # phi window (95 wide)
    cb3c = P("cb3c", (NI, L))
    e1hiT = P("e1hiT", (L - 1, H, H))         # f32 (A matmul)
    e1hjT = P("e1hjT", (L - 1, H, H), FP16)
    e1drep = P("e1drep", (L - 1, 16 * H), FP16)
    ones6k = P("ones6k", (1, 16 * NJ), FP16)
    eb1 = P("eb1", (H, L - 1))
    e2T = P("e2T", (L - 1, H, H), FP16)
    eb2 = P("eb2", (H, L - 1))
    attw = P("attw", (L - 1, H, 2 * SUB - 1), FP16)  # att window (23 wide)
    nw1hT = P("nw1hT", (L - 1, H, H))
    nw1mT = P("nw1mT", (L - 1, H, H))
    nb1 = P("nb1", (H, L - 1))
    nw2T = P("nw2T", (L - 1, H, H))
    nb2 = P("nb2", (H, L - 1))
    ones128 = P("ones128", (1, H), FP16)

    o_x = nc.declare_dram_parameter("o_x", [N, D], F32, isOutput=True)

    # collective bounce buffers (h in fp16; x in f32)
    hag_in = [nc.dram_tensor(f"hag_in{l}", [H, NI], FP16) for l in range(L)]
    hag_out = [nc.dram_tensor(f"hag_out{l}", [NC * H, NI], FP16,
                              addr_space="Shared") for l in range(L)]
    hag_h = [[nc.dram_tensor(f"hag_h{l}_{h}", [H, 24], FP16)
              for h in range(2)] for l in range(L)]
    hag_ho = [[nc.dram_tensor(f"hag_ho{l}_{h}", [NC * H, 24], FP16,
                              addr_space="Shared") for h in range(2)]
              for l in range(L)]
    xag_in = [nc.dram_tensor(f"xag_in{l}", [D, NI], F32) for l in range(L - 1)]
    xag_in.append(nc.dram_tensor(f"xag_in3", [NI, D], F32))
    xag_out = [nc.dram_tensor(f"xag_out{l}", [NC, D, NI], F32, addr_space="Shared")
               for l in range(L - 1)]
    xag_out.append(nc.dram_tensor(f"xag_out3", [N, D], F32, addr_space="Shared"))
    rg = [list(range(NC))]
    wag_in = nc.dram_tensor("wag_in", [1, 8], FP16)
    wag_out = nc.dram_tensor("wag_out", [NC, 8], FP16, addr_space="Shared")
    hag_q = [nc.dram_tensor(f"hag_q{q}", [H, 12], FP16) for q in range(4)]
    hag_qo = [nc.dram_tensor(f"hag_qo{q}", [NC * H, 12], FP16,
                             addr_space="Shared") for q in range(4)]

    with tile.TileContext(nc) as tc:
        with (
            tc.tile_pool(name="consts", bufs=1) as consts,
            tc.tile_pool(name="embp", bufs=2) as embp,
            tc.tile_pool(name="work", bufs=2) as work,
            tc.tile_pool(name="slab", bufs=1) as slabp,
            tc.tile_pool(name="cp", bufs=2) as cp,
            tc.tile_pool(name="mgp", bufs=2) as mgp,
            tc.tile_pool(name="ps_mlp", bufs=2, space="PSUM") as ps_mlp,
            tc.tile_pool(name="ps_acc", bufs=2, space="PSUM") as ps_acc,
        ):
            # warm up the collective path so the first real AllGather
            # doesn't pay the cold-start cost
            wz = cp.tile([1, 8], FP16, tag="wz")
            nc.vector.memset(wz, 0.0)
            nc.sync.dma_start(out=wag_in[:], in_=wz)
            nc.gpsimd.collective_compute(
                "AllGather", OP.bypass, replica_groups=rg,
                ins=[wag_in[:]], outs=[wag_out[:]],
            )

            # ---- embedding row-sum first: its DMAs dominate the
            # startup critical path, so issue them before the const loads
            embbT_sb = consts.tile([H, NI], F32, tag="embbT")
            nc.scalar.dma_start(out=embbT_sb, in_=embbT[:])

            h_my = cp.tile([H, NI], F32, tag="hmy")

            EB = 2  # nodes per reduce; each node block is one dense DMA
            for t in range(NI // EB):
                et = embp.tile([H, EB, NJ], F32, tag="embt")
                for a in range(EB):
                    n = EB * t + a
                    eng = nc.sync if n % 2 == 0 else nc.scalar
                    eng.dma_start(out=et[:, a:a + 1, :],
                                  in_=embw[n * H:(n + 1) * H, :])
                nc.vector.tensor_reduce(
                    out=h_my[:, t * EB:(t + 1) * EB], in_=et,
                    axis=mybir.AxisListType.X, op=OP.add,
                )

            # ---- load constants (cycled over queues) ----
            _ldq = [0]

            def load(pname, ap_in, shape, dt=F32):
                t = consts.tile(list(shape), dt, tag=pname)
                eng = [nc.scalar, nc.sync][_ldq[0] % 2]
                _ldq[0] += 1
                eng.dma_start(out=t, in_=ap_in)
                return t

            c1hiT_sb = load("c1hiT", c1hiT.rearrange("l p x -> p l x"), (H, L, H))
            c1hjT_sb = load("c1hjT", c1hjT.rearrange("l p x -> p l x"), (H, L, H),
                            FP16)
            c2T_sb = load("c2T", c2T.rearrange("l p x -> p l x"), (H, L, H), FP16)
            c3w_sb = load("c3w", c3w.rearrange("l p x -> p l x"),
                          (H, L, 2 * NI - 1), FP16)
            cb1_sb = load("cb1", cb1[:], (H, L))
            cb2_sb = load("cb2", cb2[:], (H, L))
            cb3c_sb = load("cb3c", cb3c[:], (NI, L))
            e1hiT_sb = load("e1hiT", e1hiT.rearrange("l p x -> p l x"),
                            (H, L - 1, H))
            e1hjT_sb = load("e1hjT", e1hjT.rearrange("l p x -> p l x"),
                            (H, L - 1, H), FP16)
            e2T_sb = load("e2T", e2T.rearrange("l p x -> p l x"), (H, L - 1, H),
                          FP16)
            attw_sb = load("attw", attw.rearrange("l p x -> p l x"),
                           (H, L - 1, 2 * SUB - 1), FP16)
            eb1_sb = load("eb1", eb1[:], (H, L - 1))
            eb2_sb = load("eb2", eb2[:], (H, L - 1))
            nw1hT_sb = load("nw1hT", nw1hT.rearrange("l p x -> p l x"),
                            (H, L - 1, H))
            nw1mT_sb = load("nw1mT", nw1mT.rearrange("l p x -> p l x"),
                            (H, L - 1, H))
            nw2T_sb = load("nw2T", nw2T.rearrange("l p x -> p l x"),
                           (H, L - 1, H))
            nb1_sb = load("nb1", nb1[:], (H, L - 1))
            nb2_sb = load("nb2", nb2[:], (H, L - 1))
            maskc_sb = load("maskc", maskc[:], (NI, NJ))
            eyec_sb = load("eyec", eyec[:], (NI, NJ))
            hm4_sb = load("hm4", hm4[:], (SUB, NSUB * NJ))
            x0my_sb = load("x0my", x0my[:], (NI, D))
            c1drep_sb = load("c1drep", c1drep[:], (L, 16 * H), FP16)
            ones_sb = load("ones128", ones128[:], (1, H), FP16)
            e1drep_sb = load("e1drep", e1drep[:], (L - 1, 16 * H), FP16)

            # d2ones ping-pong (row 32k: d2 strip k; row 32k+1: ones)
            d2o = [consts.tile([66, 16 * NJ], FP16, tag=f"d2o{k}",
                               name=f"d2o{k}")
                   for k in range(2)]
            for t in d2o:
                for k in range(3):
                    nc.sync.dma_start(out=t[32 * k + 1:32 * k + 2, :],
                                      in_=ones6k[:])
            combC = consts.tile([66, 16 * H], FP16, tag="combC")
            combE = consts.tile([66, 16 * H], FP16, tag="combE")

            # ---- phase 0 tail: bias add + single initial h-AG ----
            nc.vector.tensor_tensor(out=h_my[:], in0=h_my, in1=embbT_sb,
                                    op=OP.add)
            hb0 = cp.tile([H, NI], FP16, tag="hagb0")
            nc.vector.tensor_copy(hb0, h_my)
            nc.sync.dma_start(out=hag_in[0][:], in_=hb0)
            nc.gpsimd.collective_compute(
                "AllGather", OP.bypass, replica_groups=rg,
                ins=[hag_in[0][:]], outs=[hag_out[0][:]],
            )

            def send_h(l, hmy_t):
                hb = cp.tile([H, NI], FP16, tag="hagb")
                nc.vector.tensor_copy(hb, hmy_t)
                nc.sync.dma_start(out=hag_in[l][:], in_=hb)
                nc.gpsimd.collective_compute(
                    "AllGather", OP.bypass, replica_groups=rg,
                    ins=[hag_in[l][:]], outs=[hag_out[l][:]],
                )

            def recv_h(l):
                hT = cp.tile([H, NJ], FP16, tag="hT")
                engs = [nc.sync, nc.gpsimd, nc.scalar]
                for r in range(NC):
                    engs[r % 3].dma_start(out=hT[:, r * NI:(r + 1) * NI],
                                          in_=hag_out[l][r * H:(r + 1) * H, :])
                return hT

            def send_h_half(l, half, hmy_t):
                # columns {0-7,16-23,32-39} (half 0) / {8-15,24-31,40-47}
                hv = hmy_t.rearrange("p (b c) -> p b c", b=3)
                hb = cp.tile([H, 3, 8], FP16, tag=f"hagb{half}")
                nc.vector.tensor_copy(hb, hv[:, :, 8 * half:8 * half + 8])
                nc.sync.dma_start(out=hag_h[l][half][:], in_=hb)
                nc.gpsimd.collective_compute(
                    "AllGather", OP.bypass, replica_groups=rg,
                    ins=[hag_h[l][half][:]], outs=[hag_ho[l][half][:]],
                )

            def recv_h_halves(l):
                hT = cp.tile([H, NJ], FP16, tag="hT")
                hv = hT.rearrange("p (rr b c) -> p rr b c", rr=NC, b=3)
                engs = [nc.sync, nc.gpsimd, nc.scalar]
                for half in range(2):
                    for r in range(NC):
                        src_ap = hag_ho[l][half][r * H:(r + 1) * H, :]
                        engs[r % 3].dma_start(
                            out=hv[:, r, :, 8 * half:8 * half + 8],
                            in_=src_ap.rearrange("p (b c) -> p b c", b=3))
                return hT

            x_my = x0my_sb

            # ---- per-layer prep: diff/d2 + fp16 d2 strips ----
            def prep_geom(l, x_t):
                diff = []
                for c in range(D):
                    xb = cp.tile([NI, NJ], F32, tag=f"xb{c}")
                    if l == 0:
                        bsrc = bass.AP(tensor=x0rows, offset=c * NJ,
                                       ap=[[0, NI], [1, NJ]])
                    else:
                        bsrc = bass.AP(tensor=xag_out[l - 1], offset=c * NI,
                                       ap=[[0, NI], [D * NI, NC], [1, NI]])
                    nc.sync.dma_start(out=xb, in_=bsrc)
                    dc = cp.tile([NI, NJ], F32, tag=f"diff{c}")
                    nc.vector.tensor_scalar(
                        out=dc, in0=xb, scalar1=x_t[:, c:c + 1], scalar2=None,
                        op0=OP.subtract,
                    )
                    diff.append(dc)
                d2 = cp.tile([NI, NJ], F32, tag="d2")
                tmp = cp.tile([NI, NJ], F32, tag="ctmp")
                nc.vector.tensor_tensor(out=d2, in0=diff[0], in1=diff[0],
                                        op=OP.mult)
                nc.vector.tensor_tensor(out=tmp, in0=diff[1], in1=diff[1],
                                        op=OP.mult)
                nc.vector.tensor_tensor(out=d2, in0=d2, in1=tmp, op=OP.add)
                nc.vector.tensor_tensor(out=tmp, in0=diff[2], in1=diff[2],
                                        op=OP.mult)
                nc.vector.tensor_tensor(out=d2, in0=d2, in1=tmp, op=OP.add)
                d2s = cp.tile([NI, NJ], F32, tag="d2s")
                nc.vector.tensor_tensor(out=d2s, in0=d2, in1=eyec_sb, op=OP.add)
                d2b = cp.tile([NI, NJ], FP16, tag="d2b")
                nc.vector.tensor_copy(d2b, d2)
                dst = d2o[l % 2]
                for k in range(3):
                    nc.sync.dma_start(out=dst[32 * k:32 * k + 1, :],
                                      in_=d2b[16 * k:16 * (k + 1), :])
                # u = 1/(1+sqrt(d2s)) off the layer-end critical path
                dn = cp.tile([NI, NJ], F32, tag="dn")
                nc.scalar.activation(out=dn, in_=d2s, func=AF.Sqrt)
                nc.vector.tensor_scalar(out=dn, in0=dn, scalar1=1.0,
                                        scalar2=None, op0=OP.add)
                u = cp.tile([NI, NJ], F32, tag="u")
                nc.vector.reciprocal(out=u, in_=dn)
                return diff, u

            def make_comb(l, hiT_l, wdrep_l, comb, hmy_t, atag):
                At_ps = ps_acc.tile([NI, H], F32, tag="acc")
                nc.tensor.matmul(At_ps, lhsT=hmy_t, rhs=hiT_l, start=True,
                                 stop=True)
                At = cp.tile([NI, H], FP16, tag=atag)
                nc.vector.tensor_copy(At, At_ps)
                for k in range(3):
                    nc.sync.dma_start(out=comb[32 * k:32 * k + 1, :],
                                      in_=wdrep_l)
                    nc.sync.dma_start(out=comb[32 * k + 1:32 * k + 2, :],
                                      in_=At[16 * k:16 * (k + 1), :])

            diff, u_t = prep_geom(0, x0my_sb)
            hT_bf = recv_h(0)
            make_comb(0, c1hiT_sb[:, 0, :], c1drep_sb[0:1, :], combC, h_my,
                      "AtC")
            make_comb(0, e1hiT_sb[:, 0, :], e1drep_sb[0:1, :], combE, h_my,
                      "AtE")

            for l in range(L):
                last = l == L - 1
                d2l = d2o[l % 2]
                hT_l = hT_bf
                hmy_l = h_my

                # one MLP group g covers rows (g, g+16, g+32): the three K=2
                # distance matmuls hit PE row-blocks 0/32/64 and run packed.
                def emit_group(comb, b1c, hjT, w2T, b2c, g, slab):
                    pre = ps_mlp.tile([H, G * 512], F32, tag="mlp")
                    for r in range(G):
                        nc.tensor.matmul(
                            pre[:, r * 512:r * 512 + NJ],
                            lhsT=comb[32 * r:32 * r + 2,
                                      g * H:(g + 1) * H],
                            rhs=d2l[32 * r:32 * r + 2,
                                    g * NJ:(g + 1) * NJ],
                            start=True, stop=False)
                    for r in range(G):
                        nc.tensor.matmul(
                            pre[:, r * 512:r * 512 + NJ],
                            lhsT=hjT, rhs=hT_l,
                            start=False, stop=True)
                    t1 = work.tile([H, G * NJ], FP16, tag="t1", bufs=3)
                    nc.scalar.activation(
                        out=t1[:, :].rearrange("p (r c) -> p r c", r=G),
                        in_=pre.rearrange("p (r c) -> p r c", r=G)[:, :, 0:NJ],
                        func=AF.Silu, bias=b1c, scale=1.0,
                    )
                    z2 = pre  # reuse the pre banks: t1 has been read out
                    nc.tensor.matmul(z2[:, 0:512], lhsT=w2T, rhs=t1[:, 0:512],
                                     start=True, stop=True)
                    nc.tensor.matmul(z2[:, 512:1024], lhsT=w2T,
                                     rhs=t1[:, 512:1024], start=True, stop=True)
                    nc.tensor.matmul(z2[:, 1024:1152], lhsT=w2T,
                                     rhs=t1[:, 1024:1152], start=True,
                                     stop=True)
                    if slab is not None:
                        t2 = slab[:, g * (G * NJ):(g + 1) * (G * NJ)]
                    else:
                        t2 = work.tile([H, G * NJ], FP16, tag="t2", bufs=3)
                    nc.scalar.activation(
                        out=t2, in_=z2[:, 0:G * NJ],
                        func=AF.Silu, bias=b2c, scale=1.0,
                    )
                    return t2

                phi_ps = ps_acc.tile([H, NJ], F32, tag="acc")
                pendC = []

                def flush_coord(item):
                    t2p, gp = item
                    for r in range(G):
                        i = _row(gp, r)
                        nc.tensor.matmul(
                            phi_ps[0:NI, :],
                            lhsT=c3w_sb[:, l, (NI - 1) - i:(2 * NI - 1) - i],
                            rhs=t2p[:, r * NJ:(r + 1) * NJ],
                            start=(gp == 0 and r == 0),
                            stop=(gp == NGRP - 1 and r == G - 1),
                        )

                xnew = cp.tile([NI, D], F32, tag="xnew")

                def x_chain():
                    u = u_t
                    phis = cp.tile([NI, NJ], F32, tag="phis")
                    nc.vector.tensor_scalar(out=phis, in0=phi_ps[0:NI, :],
                                            scalar1=cb3c_sb[:, l:l + 1],
                                            scalar2=None, op0=OP.add)
                    s = cp.tile([NI, NJ], F32, tag="s")
                    nc.vector.tensor_tensor(out=s, in0=phis, in1=u, op=OP.mult)
                    nc.vector.tensor_tensor(out=s, in0=s, in1=maskc_sb,
                                            op=OP.mult)
                    for c in range(D):
                        xm = mgp.tile([NI, NJ], F32, tag="xm")
                        xcol = cp.tile([NI, 1], F32, tag=f"xcol{c}")
                        nc.vector.scalar_tensor_tensor(
                            out=xm, in0=diff[c], scalar=1.0, in1=s,
                            op0=OP.mult, op1=OP.mult, accum_out=xcol)
                        nc.vector.tensor_tensor(out=xnew[:, c:c + 1], in0=xcol,
                                                in1=x_my[:, c:c + 1],
                                                op=OP.add)
                    if not last:
                        nc.sync.dma_start(out=xag_in[l]
                                          .rearrange("c n -> n c"), in_=xnew)
                    else:
                        nc.sync.dma_start(out=xag_in[l][:], in_=xnew)
                    nc.gpsimd.collective_compute(
                        "AllGather", OP.bypass, replica_groups=rg,
                        ins=[xag_in[l][:]], outs=[xag_out[l][:]],
                    )

                if last:
                    for g in range(NGRP):
                        t2c = emit_group(combC, cb1_sb[:, l:l + 1],
                                         c1hjT_sb[:, l, :], c2T_sb[:, l, :],
                                         cb2_sb[:, l:l + 1], g, None)
                        pendC.append((t2c, g))
                        if len(pendC) > 1:
                            flush_coord(pendC.pop(0))
                    while pendC:
                        flush_coord(pendC.pop(0))
                    x_chain()
                    nc.sync.dma_start(out=o_x[:], in_=xag_out[l][:])
                    break

                # ---------- merged coord+edge phase ----------
                m2slab = slabp.tile([H, NI * NJ], FP16, tag="m2")
                msumT = cp.tile([H, NI], F32, tag="msumT")
                att_sub = [None] * NSUB
                gmb = [None] * NSUB
                pendE = []
                subs_ready = [0]
                subs_done = [0]

                def sigma_sub(k):
                    tt = cp.tile([SUB, NJ], F32, tag="tt")
                    nc.scalar.activation(out=tt, in_=att_sub[k][0:SUB, :],
                                         func=AF.Tanh, scale=0.5)
                    gm = cp.tile([SUB, NJ], FP16, tag="gmb")
                    nc.vector.scalar_tensor_tensor(
                        out=gm, in0=tt, scalar=1.0,
                        in1=hm4_sb[:, k * NJ:(k + 1) * NJ],
                        op0=OP.add, op1=OP.mult)
                    gmb[k] = gm
                    subs_ready[0] += 1

                def flush_edge(gp):
                    t2p = m2slab[:, gp * (G * NJ):(gp + 1) * (G * NJ)]
                    k = gp // 4
                    for r in range(G):
                        slot = (gp % 4) + 4 * r
                        if gp % 4 == 0 and r == 0:
                            att_sub[k] = ps_acc.tile([SUB, NJ], F32,
                                                     tag="acc",
                                                     name=f"attsub{k}")
                        nc.tensor.matmul(
                            att_sub[k][:, :],
                            lhsT=attw_sb[:, l, (SUB - 1) - slot:
                                         (2 * SUB - 1) - slot],
                            rhs=t2p[:, r * NJ:(r + 1) * NJ],
                            start=(gp % 4 == 0 and r == 0),
                            stop=(gp % 4 == 3 and r == G - 1),
                        )
                    if gp % 4 == 3:
                        sigma_sub(k)

                def msum_sub(k):
                    # collapse the 12 gate rows to one partition, then
                    # broadcast each across partitions via a K=1 matmul
                    growc = mgp.tile([1, SUB * NJ], FP16, tag="growc",
                                     bufs=2)
                    nc.sync.dma_start(out=growc, in_=gmb[k])
                    for s in range(SUB):
                        i = _slot_to_row(k, s)
                        off = (i % 16) * (G * NJ) + (i // 16) * NJ
                        gb = ps_acc.tile([H, NJ], F32, tag="acc", name="gb")
                        nc.tensor.matmul(
                            gb, lhsT=ones_sb,
                            rhs=growc[0:1, s * NJ:(s + 1) * NJ],
                            start=True, stop=True)
                        mg = mgp.tile([H, NJ], FP16, tag="mg")
                        nc.vector.scalar_tensor_tensor(
                            out=mg, in0=m2slab[:, off:off + NJ],
                            scalar=1.0, in1=gb,
                            op0=OP.mult, op1=OP.mult,
                            accum_out=msumT[:, i:i + 1])

                h_my_new = cp.tile([H, NI], F32, tag="hmy")

                def node_half(half):
                    # node MLP on 24 local nodes: cols {0-7,16-23,32-39}+8*half
                    sl = slice(8 * half, 8 * half + 8)
                    hv = hmy_l.rearrange("p (b c) -> p b c", b=3)
                    mv = msumT.rearrange("p (b c) -> p b c", b=3)
                    z1m = ps_acc.tile([H, 24], F32, tag="acc",
                                      name=f"z1m{half}")
                    nc.tensor.matmul(z1m, lhsT=nw1hT_sb[:, l, :],
                                     rhs=hv[:, :, sl], start=True, stop=False)
                    nc.tensor.matmul(z1m, lhsT=nw1mT_sb[:, l, :],
                                     rhs=mv[:, :, sl], start=False, stop=True)
                    t1nm = cp.tile([H, 24], F32, tag=f"t1nm{half}")
                    nc.scalar.activation(out=t1nm, in_=z1m, func=AF.Silu,
                                         bias=nb1_sb[:, l:l + 1], scale=1.0)
                    z2m = ps_acc.tile([H, 24], F32, tag="acc",
                                      name=f"z2m{half}")
                    nc.tensor.matmul(z2m, lhsT=nw2T_sb[:, l, :], rhs=t1nm,
                                     start=True, stop=True)
                    nv = h_my_new.rearrange("p (b c) -> p b c", b=3)
                    nc.vector.tensor_scalar(out=nv[:, :, sl], in0=z2m,
                                            scalar1=nb2_sb[:, l:l + 1],
                                            scalar2=None, op0=OP.add)
                    send_h_half(l + 1, half, h_my_new)

                # slot schedule: 2 coord : 1 edge until coords exhausted,
                # then the remaining edge groups
                slots = []
                for q in range(8):
                    slots += ["C", "C", "E"]
                slots += ["E"] * 8
                ci = [0]
                ei = [0]
                xdone = [False]
                prep_at = [None]

                for si, kind in enumerate(slots):
                    if kind == "C":
                        t2c = emit_group(combC, cb1_sb[:, l:l + 1],
                                         c1hjT_sb[:, l, :], c2T_sb[:, l, :],
                                         cb2_sb[:, l:l + 1], ci[0], None)
                        pendC.append((t2c, ci[0]))
                        ci[0] += 1
                        if len(pendC) > 1:
                            flush_coord(pendC.pop(0))
                    else:
                        emit_group(combE, eb1_sb[:, l:l + 1],
                                   e1hjT_sb[:, l, :], e2T_sb[:, l, :],
                                   eb2_sb[:, l:l + 1], ei[0], m2slab)
                        pendE.append(ei[0])
                        ei[0] += 1
                        if len(pendE) > 1:
                            flush_edge(pendE.pop(0))
                    if ci[0] == NGRP and not xdone[0]:
                        while pendC:
                            flush_coord(pendC.pop(0))
                        x_chain()
                        xdone[0] = True
                        prep_at[0] = si + 3
                    if prep_at[0] is not None and si == prep_at[0]:
                        diff, u_t = prep_geom(l + 1, xnew)
                    while xdone[0] and subs_done[0] < subs_ready[0]:
                        msum_sub(subs_done[0])
                        subs_done[0] += 1
                        if subs_done[0] == 2:
                            node_half(0)
                while pendE:
                    flush_edge(pendE.pop(0))
                while subs_done[0] < subs_ready[0]:
                    msum_sub(subs_done[0])
                    subs_done[0] += 1
                    if subs_done[0] == 2:
                        node_half(0)
                node_half(1)
                h_my = h_my_new

                # prep for next layer while the h AllGather flies
                make_comb(l + 1, c1hiT_sb[:, l + 1, :],
                          c1drep_sb[l + 1:l + 2, :], combC, h_my, "AtC")
                if l + 1 < L - 1:
                    make_comb(l + 1, e1hiT_sb[:, l + 1, :],
                              e1drep_sb[l + 1:l + 2, :], combE, h_my, "AtE")
                x_my = xnew
                hT_bf = recv_h_halves(l + 1)

    nc.finalize()
    return nc


def _prep_inputs(inputs):
    """Host-side prep: per-core input maps from full arrays."""
    F16 = np.float16
    f = lambda a: np.ascontiguousarray(np.asarray(a), dtype=np.float32)
    b = lambda a: np.ascontiguousarray(np.asarray(a, dtype=np.float32)
                                       .astype(F16))
    x_inp = f(inputs["x_inp"])
    emb_w = f(inputs["emb_w"])
    emb_b = f(inputs["emb_b"])
    coord_w1 = f(inputs["coord_w1"])
    coord_b1 = f(inputs["coord_b1"])
    coord_w2 = f(inputs["coord_w2"])
    coord_b2 = f(inputs["coord_b2"])
    coord_w3 = f(inputs["coord_w3"])
    coord_b3 = f(inputs["coord_b3"])
    edge_w1 = f(inputs["edge_w1"])
    edge_b1 = f(inputs["edge_b1"])
    edge_w2 = f(inputs["edge_w2"])
    edge_b2 = f(inputs["edge_b2"])
    node_w1 = f(inputs["node_w1"])
    node_b1 = f(inputs["node_b1"])
    node_w2 = f(inputs["node_w2"])
    node_b2 = f(inputs["node_b2"])
    att_w = f(inputs["att_w"])

    x0 = x_inp.reshape(N, D)
    eye = np.eye(N, dtype=np.float32)

    def stackT(w, lo, hi):
        return np.ascontiguousarray(
            np.stack([w[l, :, lo:hi].T for l in range(w.shape[0])]))

    def win(w3, S):
        nl = w3.shape[0]
        out = np.zeros((nl, H, 2 * S - 1), np.float32)
        out[:, :, S - 1] = w3[:, 0, :]
        return out

    shared = dict(
        x0rows=np.ascontiguousarray(x0.T.reshape(1, D * N)),
        c1hiT=stackT(coord_w1, 0, H),
        c1hjT=b(stackT(coord_w1, H, 2 * H)),
        c1drep=b(np.tile(coord_w1[:, :, 2 * H], (1, 16))),
        cb1=np.ascontiguousarray(coord_b1.T),
        c2T=b(np.stack([coord_w2[l].T for l in range(L)])),
        cb2=np.ascontiguousarray(coord_b2.T),
        c3w=b(win(coord_w3, NI)),
        cb3c=np.ascontiguousarray(
            np.broadcast_to(coord_b3[:, 0][None, :], (NI, L))),
        e1hiT=stackT(edge_w1, 0, H),
        e1hjT=b(stackT(edge_w1, H, 2 * H)),
        e1drep=b(np.tile(edge_w1[:, :, 2 * H], (1, 16))),
        ones6k=b(np.ones((1, 16 * NJ), np.float32)),
        eb1=np.ascontiguousarray(edge_b1.T),
        e2T=b(np.stack([edge_w2[l].T for l in range(L - 1)])),
        eb2=np.ascontiguousarray(edge_b2.T),
        attw=b(win(att_w, SUB)),
        nw1hT=stackT(node_w1, 0, H),
        nw1mT=stackT(node_w1, H, 2 * H),
        nb1=np.ascontiguousarray(node_b1.T),
        nw2T=np.ascontiguousarray(np.stack([node_w2[l].T
                                            for l in range(L - 1)])),
        nb2=np.ascontiguousarray(node_b2.T),
        ones128=b(np.ones((1, H), np.float32)),
    )
    in_maps = []
    for c in range(NC):
        m = dict(shared)
        m["embw"] = np.ascontiguousarray(
            emb_w[c * EMB_ROWS:(c + 1) * EMB_ROWS, :])
        m["embbT"] = np.ascontiguousarray(
            emb_b[c * EMB_ROWS:(c + 1) * EMB_ROWS].reshape(NI, H).T)
        m["x0my"] = np.ascontiguousarray(x0[c * NI:(c + 1) * NI, :])
        mk = 1.0 - eye[c * NI:(c + 1) * NI, :]
        m["maskc"] = np.ascontiguousarray(mk)
        m["eyec"] = np.ascontiguousarray(eye[c * NI:(c + 1) * NI, :])
        # 0.5*mask rows in (sub, slot) layout
        hm = np.zeros((SUB, NSUB * NJ), np.float32)
        for k in range(NSUB):
            for s in range(SUB):
                hm[s, k * NJ:(k + 1) * NJ] = 0.5 * mk[_slot_to_row(k, s), :]
        m["hm4"] = np.ascontiguousarray(hm)
        in_maps.append(m)
    return in_maps


def _run(inputs, trace=False, **kw):
    from concourse.bass_utils import run_bass_kernel_spmd
    if "nc" not in _cache:
        _cache["nc"] = _build_nc()
    in_maps = _prep_inputs(inputs)
    return run_bass_kernel_spmd(_cache["nc"], in_maps, list(range(NC)),
                                trace=trace, **kw)


def kernel(**inputs) -> np.ndarray:
    res = _run(inputs)
    return np.asarray(res.results[0]["o_x"], dtype=np.float32).reshape(N * D)


# revision 35
# speedup vs baseline: 1.0621x; 1.0116x over previous
"""EGNN (N=384, D=3, H=128, L=4) Bass kernel for 8 TRN2 NeuronCores. v3.

Sharding: rows of the N x N edge grid split across 8 cores (48 rows each).
Per layer: coord MLP first (phi -> x update -> x AllGather issued mid-layer),
then edge MLP; gated message row-sums (msum) overlap the remaining edge
groups via 12-row attention sub-accumulators. The node MLP runs on the LOCAL
48 nodes only and the new h rows are AllGathered in fp16.

Edge-grid matmuls run in fp16 (fp32 PSUM accumulation) - fp16 keeps weight
quantization ~1e-3 where bf16 was ~1e-2. MLP groups take rows (g, g+16,
g+32) so the three K=2 distance matmuls of a group occupy PE row-blocks
0/32/64 and execute concurrently (tile-position row packing). The gated
message multiply uses a gpsimd partition_broadcast of the gate rows instead
of a PE broadcast matmul. Sigmoid is expressed via tanh to stay in the silu
ACT table set.

Self-contained: shapes hardcoded, inputs are the full unsharded arrays.
"""
import numpy as np

N, D, H, L = 384, 3, 128, 4
NC = 8
NI = N // NC          # 48 rows per core
NJ = N                # 384 cols
G = 3                 # i-rows per MLP group (rows g, g+16, g+32)
NGRP = 16             # groups per pass
SUB = 12              # attention sub-accumulator rows (4 groups each)
NSUB = 4
EMB_ROWS = N * H // NC  # 6144 emb_w rows per core

_cache = {}


def _row(g, r):
    return g + 16 * r


def _slot_to_row(k, s):
    # slot s of sub k -> actual row index
    return (4 * k + s % 4) + 16 * (s // 4)


def _build_nc():
    import concourse.bass as bass
    import concourse.bacc as bacc
    import concourse.tile as tile
    from concourse import mybir

    F32 = mybir.dt.float32
    FP16 = mybir.dt.float16
    AF = mybir.ActivationFunctionType
    OP = mybir.AluOpType

    nc = bacc.Bacc(None, target_bir_lowering=False)

    def P(name, shape, dt=F32):
        return nc.declare_dram_parameter(name, list(shape), dt, isOutput=False)

    # per-core inputs
    embw = P("embw", (EMB_ROWS, NJ))
    embbT = P("embbT", (H, NI))
    x0my = P("x0my", (NI, D))
    maskc = P("maskc", (NI, NJ))
    eyec = P("eyec", (NI, NJ))
    hm4 = P("hm4", (SUB, NSUB * NJ))          # 0.5*mask rows, slot layout
    # shared inputs (fp16 weights for the edge-grid MLP passes)
    x0rows = P("x0rows", (1, D * NJ))
    c1hiT = P("c1hiT", (L, H, H))             # f32 (A matmul)
    c1hjT = P("c1hjT", (L, H, H), FP16)
    c1drep = P("c1drep", (L, 16 * H), FP16)
    cb1 = P("cb1", (H, L))
    c2T = P("c2T", (L, H, H), FP16)
    cb2 = P("cb2", (H, L))
    c3w = P("c3w", (L, H, 2 * NI - 1), FP16)  # phi window (95 wide)
    cb3c = P("cb3c", (NI, L))
    e1hiT = P("e1hiT", (L - 1, H, H))         # f32 (A matmul)
    e1hjT = P("e1hjT", (L - 1, H, H), FP16)
    e1drep = P("e1drep", (L - 1, 16 * H), FP16)
    ones6k = P("ones6k", (1, 16 * NJ), FP16)
    eb1 = P("eb1", (H, L - 1))
    e2T = P("e2T", (L - 1, H, H), FP16)
    eb2 = P("eb2", (H, L - 1))
    attw = P("attw", (L - 1, H, 2 * SUB - 1), FP16)  # att window (23 wide)
    nw1hT = P("nw1hT", (L - 1, H, H))
    nw1mT = P("nw1mT", (L - 1, H, H))
    nb1 = P("nb1", (H, L - 1))
    nw2T = P("nw2T", (L - 1, H, H))
    nb2 = P("nb2", (H, L - 1))
    ones128 = P("ones128", (1, H), FP16)

    o_x = nc.declare_dram_parameter("o_x", [N, D], F32, isOutput=True)

    # collective bounce buffers (h in fp16; x in f32)
    hag_in = [nc.dram_tensor(f"hag_in{l}", [H, NI], FP16) for l in range(L)]
    hag_out = [nc.dram_tensor(f"hag_out{l}", [NC * H, NI], FP16,
                              addr_space="Shared") for l in range(L)]
    hag_h = [[nc.dram_tensor(f"hag_h{l}_{h}", [H, 24], FP16)
              for h in range(2)] for l in range(L)]
    hag_ho = [[nc.dram_tensor(f"hag_ho{l}_{h}", [NC * H, 24], FP16,
                              addr_space="Shared") for h in range(2)]
              for l in range(L)]
    xag_in = [nc.dram_tensor(f"xag_in{l}", [D, NI], F32) for l in range(L - 1)]
    xag_in.append(nc.dram_tensor(f"xag_in3", [NI, D], F32))
    xag_out = [nc.dram_tensor(f"xag_out{l}", [NC, D, NI], F32, addr_space="Shared")
               for l in range(L - 1)]
    xag_out.append(nc.dram_tensor(f"xag_out3", [N, D], F32, addr_space="Shared"))
    rg = [list(range(NC))]
    wag_in = nc.dram_tensor("wag_in", [1, 8], FP16)
    wag_out = nc.dram_tensor("wag_out", [NC, 8], FP16, addr_space="Shared")
    hag_q = [nc.dram_tensor(f"hag_q{q}", [H, 12], FP16) for q in range(4)]
    hag_qo = [nc.dram_tensor(f"hag_qo{q}", [NC * H, 12], FP16,
                             addr_space="Shared") for q in range(4)]

    with tile.TileContext(nc) as tc:
        with (
            tc.tile_pool(name="consts", bufs=1) as consts,
            tc.tile_pool(name="embp", bufs=2) as embp,
            tc.tile_pool(name="work", bufs=2) as work,
            tc.tile_pool(name="slab", bufs=1) as slabp,
            tc.tile_pool(name="cp", bufs=2) as cp,
            tc.tile_pool(name="mgp", bufs=2) as mgp,
            tc.tile_pool(name="ps_mlp", bufs=2, space="PSUM") as ps_mlp,
            tc.tile_pool(name="ps_acc", bufs=2, space="PSUM") as ps_acc,
        ):
            # warm up the collective path so the first real AllGather
            # doesn't pay the cold-start cost
            wz = cp.tile([1, 8], FP16, tag="wz")
            nc.vector.memset(wz, 0.0)
            nc.sync.dma_start(out=wag_in[:], in_=wz)
            nc.gpsimd.collective_compute(
                "AllGather", OP.bypass, replica_groups=rg,
                ins=[wag_in[:]], outs=[wag_out[:]],
            )

            # ---- embedding row-sum first: its DMAs dominate the
            # startup critical path, so issue them before the const loads
            embbT_sb = consts.tile([H, NI], F32, tag="embbT")
            nc.scalar.dma_start(out=embbT_sb, in_=embbT[:])

            h_my = cp.tile([H, NI], F32, tag="hmy")

            EB = 2  # nodes per reduce; each node block is one dense DMA
            for t in range(NI // EB):
                et = embp.tile([H, EB, NJ], F32, tag="embt")
                for a in range(EB):
                    n = EB * t + a
                    eng = nc.sync if n % 2 == 0 else nc.scalar
                    eng.dma_start(out=et[:, a:a + 1, :],
                                  in_=embw[n * H:(n + 1) * H, :])
                nc.vector.tensor_reduce(
                    out=h_my[:, t * EB:(t + 1) * EB], in_=et,
                    axis=mybir.AxisListType.X, op=OP.add,
                )

            # ---- load constants (cycled over queues) ----
            _ldq = [0]

            def load(pname, ap_in, shape, dt=F32):
                t = consts.tile(list(shape), dt, tag=pname)
                eng = [nc.scalar, nc.sync][_ldq[0] % 2]
                _ldq[0] += 1
                eng.dma_start(out=t, in_=ap_in)
                return t

            c1hiT_sb = load("c1hiT", c1hiT.rearrange("l p x -> p l x"), (H, L, H))
            c1hjT_sb = load("c1hjT", c1hjT.rearrange("l p x -> p l x"), (H, L, H),
                            FP16)
            c2T_sb = load("c2T", c2T.rearrange("l p x -> p l x"), (H, L, H), FP16)
            c3w_sb = load("c3w", c3w.rearrange("l p x -> p l x"),
                          (H, L, 2 * NI - 1), FP16)
            cb1_sb = load("cb1", cb1[:], (H, L))
            cb2_sb = load("cb2", cb2[:], (H, L))
            cb3c_sb = load("cb3c", cb3c[:], (NI, L))
            e1hiT_sb = load("e1hiT", e1hiT.rearrange("l p x -> p l x"),
                            (H, L - 1, H))
            e1hjT_sb = load("e1hjT", e1hjT.rearrange("l p x -> p l x"),
                            (H, L - 1, H), FP16)
            e2T_sb = load("e2T", e2T.rearrange("l p x -> p l x"), (H, L - 1, H),
                          FP16)
            attw_sb = load("attw", attw.rearrange("l p x -> p l x"),
                           (H, L - 1, 2 * SUB - 1), FP16)
            eb1_sb = load("eb1", eb1[:], (H, L - 1))
            eb2_sb = load("eb2", eb2[:], (H, L - 1))
            nw1hT_sb = load("nw1hT", nw1hT.rearrange("l p x -> p l x"),
                            (H, L - 1, H))
            nw1mT_sb = load("nw1mT", nw1mT.rearrange("l p x -> p l x"),
                            (H, L - 1, H))
            nw2T_sb = load("nw2T", nw2T.rearrange("l p x -> p l x"),
                           (H, L - 1, H))
            nb1_sb = load("nb1", nb1[:], (H, L - 1))
            nb2_sb = load("nb2", nb2[:], (H, L - 1))
            maskc_sb = load("maskc", maskc[:], (NI, NJ))
            eyec_sb = load("eyec", eyec[:], (NI, NJ))
            hm4_sb = load("hm4", hm4[:], (SUB, NSUB * NJ))
            x0my_sb = load("x0my", x0my[:], (NI, D))
            c1drep_sb = load("c1drep", c1drep[:], (L, 16 * H), FP16)
            ones_sb = load("ones128", ones128[:], (1, H), FP16)
            e1drep_sb = load("e1drep", e1drep[:], (L - 1, 16 * H), FP16)

            # d2ones ping-pong (row 32k: d2 strip k; row 32k+1: ones)
            d2o = [consts.tile([66, 16 * NJ], FP16, tag=f"d2o{k}",
                               name=f"d2o{k}")
                   for k in range(2)]
            for t in d2o:
                for k in range(3):
                    nc.sync.dma_start(out=t[32 * k + 1:32 * k + 2, :],
                                      in_=ones6k[:])
            combC = consts.tile([66, 16 * H], FP16, tag="combC")
            combE = consts.tile([66, 16 * H], FP16, tag="combE")

            # ---- phase 0 tail: bias add + single initial h-AG ----
            nc.vector.tensor_tensor(out=h_my[:], in0=h_my, in1=embbT_sb,
                                    op=OP.add)
            hb0 = cp.tile([H, NI], FP16, tag="hagb0")
            nc.vector.tensor_copy(hb0, h_my)
            nc.sync.dma_start(out=hag_in[0][:], in_=hb0)
            nc.gpsimd.collective_compute(
                "AllGather", OP.bypass, replica_groups=rg,
                ins=[hag_in[0][:]], outs=[hag_out[0][:]],
            )

            def send_h(l, hmy_t):
                hb = cp.tile([H, NI], FP16, tag="hagb")
                nc.vector.tensor_copy(hb, hmy_t)
                nc.sync.dma_start(out=hag_in[l][:], in_=hb)
                nc.gpsimd.collective_compute(
                    "AllGather", OP.bypass, replica_groups=rg,
                    ins=[hag_in[l][:]], outs=[hag_out[l][:]],
                )

            def recv_h(l):
                hT = cp.tile([H, NJ], FP16, tag="hT")
                engs = [nc.sync, nc.gpsimd, nc.scalar]
                for r in range(NC):
                    engs[r % 3].dma_start(out=hT[:, r * NI:(r + 1) * NI],
                                          in_=hag_out[l][r * H:(r + 1) * H, :])
                return hT

            def send_h_half(l, half, hmy_t):
                # columns {0-7,16-23,32-39} (half 0) / {8-15,24-31,40-47}
                hv = hmy_t.rearrange("p (b c) -> p b c", b=3)
                hb = cp.tile([H, 3, 8], FP16, tag=f"hagb{half}")
                nc.vector.tensor_copy(hb, hv[:, :, 8 * half:8 * half + 8])
                nc.sync.dma_start(out=hag_h[l][half][:], in_=hb)
                nc.gpsimd.collective_compute(
                    "AllGather", OP.bypass, replica_groups=rg,
                    ins=[hag_h[l][half][:]], outs=[hag_ho[l][half][:]],
                )

            def recv_h_halves(l):
                hT = cp.tile([H, NJ], FP16, tag="hT")
                hv = hT.rearrange("p (rr b c) -> p rr b c", rr=NC, b=3)
                engs = [nc.sync, nc.gpsimd, nc.scalar]
                for half in range(2):
                    for r in range(NC):
                        src_ap = hag_ho[l][half][r * H:(r + 1) * H, :]
                        engs[r % 3].dma_start(
                            out=hv[:, r, :, 8 * half:8 * half + 8],
                            in_=src_ap.rearrange("p (b c) -> p b c", b=3))
                return hT

            x_my = x0my_sb

            # ---- per-layer prep: diff/d2 + fp16 d2 strips ----
            def prep_geom(l, x_t):
                diff = []
                for c in range(D):
                    xb = cp.tile([NI, NJ], F32, tag=f"xb{c}")
                    if l == 0:
                        bsrc = bass.AP(tensor=x0rows, offset=c * NJ,
                                       ap=[[0, NI], [1, NJ]])
                    else:
                        bsrc = bass.AP(tensor=xag_out[l - 1], offset=c * NI,
                                       ap=[[0, NI], [D * NI, NC], [1, NI]])
                    nc.sync.dma_start(out=xb, in_=bsrc)
                    dc = cp.tile([NI, NJ], F32, tag=f"diff{c}")
                    nc.vector.tensor_scalar(
                        out=dc, in0=xb, scalar1=x_t[:, c:c + 1], scalar2=None,
                        op0=OP.subtract,
                    )
                    diff.append(dc)
                d2 = cp.tile([NI, NJ], F32, tag="d2")
                tmp = cp.tile([NI, NJ], F32, tag="ctmp")
                nc.vector.tensor_tensor(out=d2, in0=diff[0], in1=diff[0],
                                        op=OP.mult)
                nc.vector.tensor_tensor(out=tmp, in0=diff[1], in1=diff[1],
                                        op=OP.mult)
                nc.vector.tensor_tensor(out=d2, in0=d2, in1=tmp, op=OP.add)
                nc.vector.tensor_tensor(out=tmp, in0=diff[2], in1=diff[2],
                                        op=OP.mult)
                nc.vector.tensor_tensor(out=d2, in0=d2, in1=tmp, op=OP.add)
                d2s = cp.tile([NI, NJ], F32, tag="d2s")
                nc.vector.tensor_tensor(out=d2s, in0=d2, in1=eyec_sb, op=OP.add)
                d2b = cp.tile([NI, NJ], FP16, tag="d2b")
                nc.vector.tensor_copy(d2b, d2)
                dst = d2o[l % 2]
                for k in range(3):
                    nc.sync.dma_start(out=dst[32 * k:32 * k + 1, :],
                                      in_=d2b[16 * k:16 * (k + 1), :])
                # u = 1/(1+sqrt(d2s)) off the layer-end critical path
                dn = cp.tile([NI, NJ], F32, tag="dn")
                nc.scalar.activation(out=dn, in_=d2s, func=AF.Sqrt)
                nc.vector.tensor_scalar(out=dn, in0=dn, scalar1=1.0,
                                        scalar2=None, op0=OP.add)
                u = cp.tile([NI, NJ], F32, tag="u")
                nc.vector.reciprocal(out=u, in_=dn)
                return diff, u

            def make_comb(l, hiT_l, wdrep_l, comb, hmy_t, atag):
                At_ps = ps_acc.tile([NI, H], F32, tag="acc")
                nc.tensor.matmul(At_ps, lhsT=hmy_t, rhs=hiT_l, start=True,
                                 stop=True)
                At = cp.tile([NI, H], FP16, tag=atag)
                nc.vector.tensor_copy(At, At_ps)
                for k in range(3):
                    nc.sync.dma_start(out=comb[32 * k:32 * k + 1, :],
                                      in_=wdrep_l)
                    nc.sync.dma_start(out=comb[32 * k + 1:32 * k + 2, :],
                                      in_=At[16 * k:16 * (k + 1), :])

            diff, u_t = prep_geom(0, x0my_sb)
            hT_bf = recv_h(0)
            make_comb(0, c1hiT_sb[:, 0, :], c1drep_sb[0:1, :], combC, h_my,
                      "AtC")
            make_comb(0, e1hiT_sb[:, 0, :], e1drep_sb[0:1, :], combE, h_my,
                      "AtE")

            for l in range(L):
                last = l == L - 1
                d2l = d2o[l % 2]
                hT_l = hT_bf
                hmy_l = h_my

                # one MLP group g covers rows (g, g+16, g+32): the three K=2
                # distance matmuls hit PE row-blocks 0/32/64 and run packed.
                def emit_group(comb, b1c, hjT, w2T, b2c, g, slab):
                    pre = ps_mlp.tile([H, G * 512], F32, tag="mlp")
                    for r in range(G):
                        nc.tensor.matmul(
                            pre[:, r * 512:r * 512 + NJ],
                            lhsT=comb[32 * r:32 * r + 2,
                                      g * H:(g + 1) * H],
                            rhs=d2l[32 * r:32 * r + 2,
                                    g * NJ:(g + 1) * NJ],
                            start=True, stop=False)
                    for r in range(G):
                        nc.tensor.matmul(
                            pre[:, r * 512:r * 512 + NJ],
                            lhsT=hjT, rhs=hT_l,
                            start=False, stop=True)
                    t1 = work.tile([H, G * NJ], FP16, tag="t1", bufs=4)
                    nc.scalar.activation(
                        out=t1[:, :].rearrange("p (r c) -> p r c", r=G),
                        in_=pre.rearrange("p (r c) -> p r c", r=G)[:, :, 0:NJ],
                        func=AF.Silu, bias=b1c, scale=1.0,
                    )
                    z2 = pre  # reuse the pre banks: t1 has been read out
                    nc.tensor.matmul(z2[:, 0:512], lhsT=w2T, rhs=t1[:, 0:512],
                                     start=True, stop=True)
                    nc.tensor.matmul(z2[:, 512:1024], lhsT=w2T,
                                     rhs=t1[:, 512:1024], start=True, stop=True)
                    nc.tensor.matmul(z2[:, 1024:1152], lhsT=w2T,
                                     rhs=t1[:, 1024:1152], start=True,
                                     stop=True)
                    if slab is not None:
                        t2 = slab[:, g * (G * NJ):(g + 1) * (G * NJ)]
                    else:
                        t2 = work.tile([H, G * NJ], FP16, tag="t2", bufs=3)
                    nc.scalar.activation(
                        out=t2, in_=z2[:, 0:G * NJ],
                        func=AF.Silu, bias=b2c, scale=1.0,
                    )
                    return t2

                phi_ps = ps_acc.tile([H, NJ], F32, tag="acc")
                pendC = []

                def flush_coord(item):
                    t2p, gp = item
                    for r in range(G):
                        i = _row(gp, r)
                        nc.tensor.matmul(
                            phi_ps[0:NI, :],
                            lhsT=c3w_sb[:, l, (NI - 1) - i:(2 * NI - 1) - i],
                            rhs=t2p[:, r * NJ:(r + 1) * NJ],
                            start=(gp == 0 and r == 0),
                            stop=(gp == NGRP - 1 and r == G - 1),
                        )

                xnew = cp.tile([NI, D], F32, tag="xnew")

                def x_chain():
                    u = u_t
                    phis = cp.tile([NI, NJ], F32, tag="phis")
                    nc.vector.tensor_scalar(out=phis, in0=phi_ps[0:NI, :],
                                            scalar1=cb3c_sb[:, l:l + 1],
                                            scalar2=None, op0=OP.add)
                    s = cp.tile([NI, NJ], F32, tag="s")
                    nc.vector.tensor_tensor(out=s, in0=phis, in1=u, op=OP.mult)
                    nc.vector.tensor_tensor(out=s, in0=s, in1=maskc_sb,
                                            op=OP.mult)
                    for c in range(D):
                        xm = mgp.tile([NI, NJ], F32, tag="xm")
                        xcol = cp.tile([NI, 1], F32, tag=f"xcol{c}")
                        nc.vector.scalar_tensor_tensor(
                            out=xm, in0=diff[c], scalar=1.0, in1=s,
                            op0=OP.mult, op1=OP.mult, accum_out=xcol)
                        nc.vector.tensor_tensor(out=xnew[:, c:c + 1], in0=xcol,
                                                in1=x_my[:, c:c + 1],
                                                op=OP.add)
                    if not last:
                        nc.sync.dma_start(out=xag_in[l]
                                          .rearrange("c n -> n c"), in_=xnew)
                    else:
                        nc.sync.dma_start(out=xag_in[l][:], in_=xnew)
                    nc.gpsimd.collective_compute(
                        "AllGather", OP.bypass, replica_groups=rg,
                        ins=[xag_in[l][:]], outs=[xag_out[l][:]],
                    )

                if last:
                    for g in range(NGRP):
                        t2c = emit_group(combC, cb1_sb[:, l:l + 1],
                                         c1hjT_sb[:, l, :], c2T_sb[:, l, :],
                                         cb2_sb[:, l:l + 1], g, None)
                        pendC.append((t2c, g))
                        if len(pendC) > 1:
                            flush_coord(pendC.pop(0))
                    while pendC:
                        flush_coord(pendC.pop(0))
                    x_chain()
                    nc.sync.dma_start(out=o_x[:], in_=xag_out[l][:])
                    break

                # ---------- merged coord+edge phase ----------
                m2slab = slabp.tile([H, NI * NJ], FP16, tag="m2")
                msumT = cp.tile([H, NI], F32, tag="msumT")
                att_sub = [None] * NSUB
                gmb = [None] * NSUB
                pendE = []
                subs_ready = [0]
                subs_done = [0]

                def sigma_sub(k):
                    tt = cp.tile([SUB, NJ], F32, tag="tt")
                    nc.scalar.activation(out=tt, in_=att_sub[k][0:SUB, :],
                                         func=AF.Tanh, scale=0.5)
                    gm = cp.tile([SUB, NJ], FP16, tag="gmb")
                    nc.vector.scalar_tensor_tensor(
                        out=gm, in0=tt, scalar=1.0,
                        in1=hm4_sb[:, k * NJ:(k + 1) * NJ],
                        op0=OP.add, op1=OP.mult)
                    gmb[k] = gm
                    subs_ready[0] += 1

                def flush_edge(gp):
                    t2p = m2slab[:, gp * (G * NJ):(gp + 1) * (G * NJ)]
                    k = gp // 4
                    for r in range(G):
                        slot = (gp % 4) + 4 * r
                        if gp % 4 == 0 and r == 0:
                            att_sub[k] = ps_acc.tile([SUB, NJ], F32,
                                                     tag="acc",
                                                     name=f"attsub{k}")
                        nc.tensor.matmul(
                            att_sub[k][:, :],
                            lhsT=attw_sb[:, l, (SUB - 1) - slot:
                                         (2 * SUB - 1) - slot],
                            rhs=t2p[:, r * NJ:(r + 1) * NJ],
                            start=(gp % 4 == 0 and r == 0),
                            stop=(gp % 4 == 3 and r == G - 1),
                        )
                    if gp % 4 == 3:
                        sigma_sub(k)

                def msum_sub(k):
                    # collapse the 12 gate rows to one partition, then
                    # broadcast each across partitions via a K=1 matmul
                    growc = mgp.tile([1, SUB * NJ], FP16, tag="growc",
                                     bufs=2)
                    nc.sync.dma_start(out=growc, in_=gmb[k])
                    for s in range(SUB):
                        i = _slot_to_row(k, s)
                        off = (i % 16) * (G * NJ) + (i // 16) * NJ
                        gb = ps_acc.tile([H, NJ], F32, tag="acc", name="gb")
                        nc.tensor.matmul(
                            gb, lhsT=ones_sb,
                            rhs=growc[0:1, s * NJ:(s + 1) * NJ],
                            start=True, stop=True)
                        mg = mgp.tile([H, NJ], FP16, tag="mg")
                        nc.vector.scalar_tensor_tensor(
                            out=mg, in0=m2slab[:, off:off + NJ],
                            scalar=1.0, in1=gb,
                            op0=OP.mult, op1=OP.mult,
                            accum_out=msumT[:, i:i + 1])

                h_my_new = cp.tile([H, NI], F32, tag="hmy")

                def node_half(half):
                    # node MLP on 24 local nodes: cols {0-7,16-23,32-39}+8*half
                    sl = slice(8 * half, 8 * half + 8)
                    hv = hmy_l.rearrange("p (b c) -> p b c", b=3)
                    mv = msumT.rearrange("p (b c) -> p b c", b=3)
                    z1m = ps_acc.tile([H, 24], F32, tag="acc",
                                      name=f"z1m{half}")
                    nc.tensor.matmul(z1m, lhsT=nw1hT_sb[:, l, :],
                                     rhs=hv[:, :, sl], start=True, stop=False)
                    nc.tensor.matmul(z1m, lhsT=nw1mT_sb[:, l, :],
                                     rhs=mv[:, :, sl], start=False, stop=True)
                    t1nm = cp.tile([H, 24], F32, tag=f"t1nm{half}")
                    nc.scalar.activation(out=t1nm, in_=z1m, func=AF.Silu,
                                         bias=nb1_sb[:, l:l + 1], scale=1.0)
                    z2m = ps_acc.tile([H, 24], F32, tag="acc",
                                      name=f"z2m{half}")
                    nc.tensor.matmul(z2m, lhsT=nw2T_sb[:, l, :], rhs=t1nm,
                                     start=True, stop=True)
                    nv = h_my_new.rearrange("p (b c) -> p b c", b=3)
                    nc.vector.tensor_scalar(out=nv[:, :, sl], in0=z2m,
                                            scalar1=nb2_sb[:, l:l + 1],
                                            scalar2=None, op0=OP.add)
                    send_h_half(l + 1, half, h_my_new)

                # slot schedule: 2 coord : 1 edge until coords exhausted,
                # then the remaining edge groups
                slots = []
                for q in range(8):
                    slots += ["C", "C", "E"]
                slots += ["E"] * 8
                ci = [0]
                ei = [0]
                xdone = [False]
                prep_at = [None]

                for si, kind in enumerate(slots):
                    if kind == "C":
                        t2c = emit_group(combC, cb1_sb[:, l:l + 1],
                                         c1hjT_sb[:, l, :], c2T_sb[:, l, :],
                                         cb2_sb[:, l:l + 1], ci[0], None)
                        pendC.append((t2c, ci[0]))
                        ci[0] += 1
                        if len(pendC) > 2:
                            flush_coord(pendC.pop(0))
                    else:
                        emit_group(combE, eb1_sb[:, l:l + 1],
                                   e1hjT_sb[:, l, :], e2T_sb[:, l, :],
                                   eb2_sb[:, l:l + 1], ei[0], m2slab)
                        pendE.append(ei[0])
                        ei[0] += 1
                        if len(pendE) > 2:
                            flush_edge(pendE.pop(0))
                    if ci[0] == NGRP and not xdone[0]:
                        while pendC:
                            flush_coord(pendC.pop(0))
                        x_chain()
                        xdone[0] = True
                        prep_at[0] = si + 3
                    if prep_at[0] is not None and si == prep_at[0]:
                        diff, u_t = prep_geom(l + 1, xnew)
                    while xdone[0] and subs_done[0] < subs_ready[0]:
                        msum_sub(subs_done[0])
                        subs_done[0] += 1
                        if subs_done[0] == 2:
                            node_half(0)
                while pendE:
                    flush_edge(pendE.pop(0))
                while subs_done[0] < subs_ready[0]:
                    msum_sub(subs_done[0])
                    subs_done[0] += 1
                    if subs_done[0] == 2:
                        node_half(0)
                node_half(1)
                h_my = h_my_new

                # prep for next layer while the h AllGather flies
                make_comb(l + 1, c1hiT_sb[:, l + 1, :],
                          c1drep_sb[l + 1:l + 2, :], combC, h_my, "AtC")
                if l + 1 < L - 1:
                    make_comb(l + 1, e1hiT_sb[:, l + 1, :],
                              e1drep_sb[l + 1:l + 2, :], combE, h_my, "AtE")
                x_my = xnew
                hT_bf = recv_h_halves(l + 1)

    nc.finalize()
    return nc


def _prep_inputs(inputs):
    """Host-side prep: per-core input maps from full arrays."""
    F16 = np.float16
    f = lambda a: np.ascontiguousarray(np.asarray(a), dtype=np.float32)
    b = lambda a: np.ascontiguousarray(np.asarray(a, dtype=np.float32)
                                       .astype(F16))
    x_inp = f(inputs["x_inp"])
    emb_w = f(inputs["emb_w"])
    emb_b = f(inputs["emb_b"])
    coord_w1 = f(inputs["coord_w1"])
    coord_b1 = f(inputs["coord_b1"])
    coord_w2 = f(inputs["coord_w2"])
    coord_b2 = f(inputs["coord_b2"])
    coord_w3 = f(inputs["coord_w3"])
    coord_b3 = f(inputs["coord_b3"])
    edge_w1 = f(inputs["edge_w1"])
    edge_b1 = f(inputs["edge_b1"])
    edge_w2 = f(inputs["edge_w2"])
    edge_b2 = f(inputs["edge_b2"])
    node_w1 = f(inputs["node_w1"])
    node_b1 = f(inputs["node_b1"])
    node_w2 = f(inputs["node_w2"])
    node_b2 = f(inputs["node_b2"])
    att_w = f(inputs["att_w"])

    x0 = x_inp.reshape(N, D)
    eye = np.eye(N, dtype=np.float32)

    def stackT(w, lo, hi):
        return np.ascontiguousarray(
            np.stack([w[l, :, lo:hi].T for l in range(w.shape[0])]))

    def win(w3, S):
        nl = w3.shape[0]
        out = np.zeros((nl, H, 2 * S - 1), np.float32)
        out[:, :, S - 1] = w3[:, 0, :]
        return out

    shared = dict(
        x0rows=np.ascontiguousarray(x0.T.reshape(1, D * N)),
        c1hiT=stackT(coord_w1, 0, H),
        c1hjT=b(stackT(coord_w1, H, 2 * H)),
        c1drep=b(np.tile(coord_w1[:, :, 2 * H], (1, 16))),
        cb1=np.ascontiguousarray(coord_b1.T),
        c2T=b(np.stack([coord_w2[l].T for l in range(L)])),
        cb2=np.ascontiguousarray(coord_b2.T),
        c3w=b(win(coord_w3, NI)),
        cb3c=np.ascontiguousarray(
            np.broadcast_to(coord_b3[:, 0][None, :], (NI, L))),
        e1hiT=stackT(edge_w1, 0, H),
        e1hjT=b(stackT(edge_w1, H, 2 * H)),
        e1drep=b(np.tile(edge_w1[:, :, 2 * H], (1, 16))),
        ones6k=b(np.ones((1, 16 * NJ), np.float32)),
        eb1=np.ascontiguousarray(edge_b1.T),
        e2T=b(np.stack([edge_w2[l].T for l in range(L - 1)])),
        eb2=np.ascontiguousarray(edge_b2.T),
        attw=b(win(att_w, SUB)),
        nw1hT=stackT(node_w1, 0, H),
        nw1mT=stackT(node_w1, H, 2 * H),
        nb1=np.ascontiguousarray(node_b1.T),
        nw2T=np.ascontiguousarray(np.stack([node_w2[l].T
                                            for l in range(L - 1)])),
        nb2=np.ascontiguousarray(node_b2.T),
        ones128=b(np.ones((1, H), np.float32)),
    )
    in_maps = []
    for c in range(NC):
        m = dict(shared)
        m["embw"] = np.ascontiguousarray(
            emb_w[c * EMB_ROWS:(c + 1) * EMB_ROWS, :])
        m["embbT"] = np.ascontiguousarray(
            emb_b[c * EMB_ROWS:(c + 1) * EMB_ROWS].reshape(NI, H).T)
        m["x0my"] = np.ascontiguousarray(x0[c * NI:(c + 1) * NI, :])
        mk = 1.0 - eye[c * NI:(c + 1) * NI, :]
        m["maskc"] = np.ascontiguousarray(mk)
        m["eyec"] = np.ascontiguousarray(eye[c * NI:(c + 1) * NI, :])
        # 0.5*mask rows in (sub, slot) layout
        hm = np.zeros((SUB, NSUB * NJ), np.float32)
        for k in range(NSUB):
            for s in range(SUB):
                hm[s, k * NJ:(k + 1) * NJ] = 0.5 * mk[_slot_to_row(k, s), :]
        m["hm4"] = np.ascontiguousarray(hm)
        in_maps.append(m)
    return in_maps


def _run(inputs, trace=False, **kw):
    from concourse.bass_utils import run_bass_kernel_spmd
    if "nc" not in _cache:
        _cache["nc"] = _build_nc()
    in_maps = _prep_inputs(inputs)
    return run_bass_kernel_spmd(_cache["nc"], in_maps, list(range(NC)),
                                trace=trace, **kw)


def kernel(**inputs) -> np.ndarray:
    res = _run(inputs)
    return np.asarray(res.results[0]["o_x"], dtype=np.float32).reshape(N * D)


# revision 36
# speedup vs baseline: 1.0907x; 1.0269x over previous
"""EGNN (N=384, D=3, H=128, L=4) Bass kernel for 8 TRN2 NeuronCores. v3.

Sharding: rows of the N x N edge grid split across 8 cores (48 rows each).
Per layer: coord MLP first (phi -> x update -> x AllGather issued mid-layer),
then edge MLP; gated message row-sums (msum) overlap the remaining edge
groups via 12-row attention sub-accumulators. The node MLP runs on the LOCAL
48 nodes only and the new h rows are AllGathered in fp16.

Edge-grid matmuls run in fp16 (fp32 PSUM accumulation) - fp16 keeps weight
quantization ~1e-3 where bf16 was ~1e-2. MLP groups take rows (g, g+16,
g+32) so the three K=2 distance matmuls of a group occupy PE row-blocks
0/32/64 and execute concurrently (tile-position row packing). The gated
message multiply uses a gpsimd partition_broadcast of the gate rows instead
of a PE broadcast matmul. Sigmoid is expressed via tanh to stay in the silu
ACT table set.

Self-contained: shapes hardcoded, inputs are the full unsharded arrays.
"""
import numpy as np

N, D, H, L = 384, 3, 128, 4
NC = 8
NI = N // NC          # 48 rows per core
NJ = N                # 384 cols
G = 3                 # i-rows per MLP group (rows g, g+16, g+32)
NGRP = 16             # groups per pass
SUB = 12              # attention sub-accumulator rows (4 groups each)
NSUB = 4
EMB_ROWS = N * H // NC  # 6144 emb_w rows per core

_cache = {}


def _row(g, r):
    return g + 16 * r


def _slot_to_row(k, s):
    # slot s of sub k -> actual row index
    return (4 * k + s % 4) + 16 * (s // 4)


def _build_nc():
    import concourse.bass as bass
    import concourse.bacc as bacc
    import concourse.tile as tile
    from concourse import mybir

    F32 = mybir.dt.float32
    FP16 = mybir.dt.float16
    AF = mybir.ActivationFunctionType
    OP = mybir.AluOpType

    nc = bacc.Bacc(None, target_bir_lowering=False)

    def P(name, shape, dt=F32):
        return nc.declare_dram_parameter(name, list(shape), dt, isOutput=False)

    # per-core inputs
    embw = P("embw", (EMB_ROWS, NJ))
    embbT = P("embbT", (H, NI))
    x0my = P("x0my", (NI, D))
    maskc = P("maskc", (NI, NJ))
    eyec = P("eyec", (NI, NJ))
    hm4 = P("hm4", (SUB, NSUB * NJ))          # 0.5*mask rows, slot layout
    # shared inputs (fp16 weights for the edge-grid MLP passes)
    x0rows = P("x0rows", (1, D * NJ))
    c1hiT = P("c1hiT", (L, H, H))             # f32 (A matmul)
    c1hjT = P("c1hjT", (L, H, H), FP16)
    c1drep = P("c1drep", (L, 16 * H), FP16)
    cb1 = P("cb1", (H, L))
    c2T = P("c2T", (L, H, H), FP16)
    cb2 = P("cb2", (H, L))
    c3w = P("c3w", (L, H, 2 * NI - 1), FP16)  # phi window (95 wide)
    cb3c = P("cb3c", (NI, L))
    e1hiT = P("e1hiT", (L - 1, H, H))         # f32 (A matmul)
    e1hjT = P("e1hjT", (L - 1, H, H), FP16)
    e1drep = P("e1drep", (L - 1, 16 * H), FP16)
    ones6k = P("ones6k", (1, 16 * NJ), FP16)
    eb1 = P("eb1", (H, L - 1))
    e2T = P("e2T", (L - 1, H, H), FP16)
    eb2 = P("eb2", (H, L - 1))
    attw = P("attw", (L - 1, H, 2 * SUB - 1), FP16)  # att window (23 wide)
    nw1hT = P("nw1hT", (L - 1, H, H))
    nw1mT = P("nw1mT", (L - 1, H, H))
    nb1 = P("nb1", (H, L - 1))
    nw2T = P("nw2T", (L - 1, H, H))
    nb2 = P("nb2", (H, L - 1))
    ones128 = P("ones128", (1, H), FP16)

    o_x = nc.declare_dram_parameter("o_x", [N, D], F32, isOutput=True)

    # collective bounce buffers (h in fp16; x in f32)
    hag_in = [nc.dram_tensor(f"hag_in{l}", [H, NI], FP16) for l in range(L)]
    hag_out = [nc.dram_tensor(f"hag_out{l}", [NC * H, NI], FP16,
                              addr_space="Shared") for l in range(L)]
    hag_h = [[nc.dram_tensor(f"hag_h{l}_{h}", [H, 24], FP16)
              for h in range(2)] for l in range(L)]
    hag_ho = [[nc.dram_tensor(f"hag_ho{l}_{h}", [NC * H, 24], FP16,
                              addr_space="Shared") for h in range(2)]
              for l in range(L)]
    xag_in = [nc.dram_tensor(f"xag_in{l}", [D, NI], F32) for l in range(L - 1)]
    xag_in.append(nc.dram_tensor(f"xag_in3", [NI, D], F32))
    xag_out = [nc.dram_tensor(f"xag_out{l}", [NC, D, NI], F32, addr_space="Shared")
               for l in range(L - 1)]
    xag_out.append(nc.dram_tensor(f"xag_out3", [N, D], F32, addr_space="Shared"))
    rg = [list(range(NC))]
    wag_in = nc.dram_tensor("wag_in", [1, 8], FP16)
    wag_out = nc.dram_tensor("wag_out", [NC, 8], FP16, addr_space="Shared")
    hag_q = [nc.dram_tensor(f"hag_q{q}", [H, 12], FP16) for q in range(4)]
    hag_qo = [nc.dram_tensor(f"hag_qo{q}", [NC * H, 12], FP16,
                             addr_space="Shared") for q in range(4)]

    with tile.TileContext(nc) as tc:
        with (
            tc.tile_pool(name="consts", bufs=1) as consts,
            tc.tile_pool(name="embp", bufs=2) as embp,
            tc.tile_pool(name="work", bufs=2) as work,
            tc.tile_pool(name="slab", bufs=1) as slabp,
            tc.tile_pool(name="cp", bufs=2) as cp,
            tc.tile_pool(name="mgp", bufs=2) as mgp,
            tc.tile_pool(name="ps_mlp", bufs=2, space="PSUM") as ps_mlp,
            tc.tile_pool(name="ps_acc", bufs=2, space="PSUM") as ps_acc,
        ):
            # warm up the collective path so the first real AllGather
            # doesn't pay the cold-start cost
            wz = cp.tile([1, 8], FP16, tag="wz")
            nc.vector.memset(wz, 0.0)
            nc.sync.dma_start(out=wag_in[:], in_=wz)
            nc.gpsimd.collective_compute(
                "AllGather", OP.bypass, replica_groups=rg,
                ins=[wag_in[:]], outs=[wag_out[:]],
            )

            # ---- embedding row-sum first: its DMAs dominate the
            # startup critical path, so issue them before the const loads
            embbT_sb = consts.tile([H, NI], F32, tag="embbT")
            nc.scalar.dma_start(out=embbT_sb, in_=embbT[:])

            h_my = cp.tile([H, NI], F32, tag="hmy")

            EB = 2  # nodes per reduce; each node block is one dense DMA
            for t in range(NI // EB):
                et = embp.tile([H, EB, NJ], F32, tag="embt")
                for a in range(EB):
                    n = EB * t + a
                    eng = nc.sync if n % 2 == 0 else nc.scalar
                    eng.dma_start(out=et[:, a:a + 1, :],
                                  in_=embw[n * H:(n + 1) * H, :])
                nc.vector.tensor_reduce(
                    out=h_my[:, t * EB:(t + 1) * EB], in_=et,
                    axis=mybir.AxisListType.X, op=OP.add,
                )

            # ---- load constants (cycled over queues) ----
            _ldq = [0]

            def load(pname, ap_in, shape, dt=F32):
                t = consts.tile(list(shape), dt, tag=pname)
                eng = [nc.scalar, nc.sync][_ldq[0] % 2]
                _ldq[0] += 1
                eng.dma_start(out=t, in_=ap_in)
                return t

            c1hiT_sb = load("c1hiT", c1hiT.rearrange("l p x -> p l x"), (H, L, H))
            c1hjT_sb = load("c1hjT", c1hjT.rearrange("l p x -> p l x"), (H, L, H),
                            FP16)
            c2T_sb = load("c2T", c2T.rearrange("l p x -> p l x"), (H, L, H), FP16)
            c3w_sb = load("c3w", c3w.rearrange("l p x -> p l x"),
                          (H, L, 2 * NI - 1), FP16)
            cb1_sb = load("cb1", cb1[:], (H, L))
            cb2_sb = load("cb2", cb2[:], (H, L))
            cb3c_sb = load("cb3c", cb3c[:], (NI, L))
            e1hiT_sb = load("e1hiT", e1hiT.rearrange("l p x -> p l x"),
                            (H, L - 1, H))
            e1hjT_sb = load("e1hjT", e1hjT.rearrange("l p x -> p l x"),
                            (H, L - 1, H), FP16)
            e2T_sb = load("e2T", e2T.rearrange("l p x -> p l x"), (H, L - 1, H),
                          FP16)
            attw_sb = load("attw", attw.rearrange("l p x -> p l x"),
                           (H, L - 1, 2 * SUB - 1), FP16)
            eb1_sb = load("eb1", eb1[:], (H, L - 1))
            eb2_sb = load("eb2", eb2[:], (H, L - 1))
            nw1hT_sb = load("nw1hT", nw1hT.rearrange("l p x -> p l x"),
                            (H, L - 1, H))
            nw1mT_sb = load("nw1mT", nw1mT.rearrange("l p x -> p l x"),
                            (H, L - 1, H))
            nw2T_sb = load("nw2T", nw2T.rearrange("l p x -> p l x"),
                           (H, L - 1, H))
            nb1_sb = load("nb1", nb1[:], (H, L - 1))
            nb2_sb = load("nb2", nb2[:], (H, L - 1))
            maskc_sb = load("maskc", maskc[:], (NI, NJ))
            eyec_sb = load("eyec", eyec[:], (NI, NJ))
            hm4_sb = load("hm4", hm4[:], (SUB, NSUB * NJ))
            x0my_sb = load("x0my", x0my[:], (NI, D))
            c1drep_sb = load("c1drep", c1drep[:], (L, 16 * H), FP16)
            ones_sb = load("ones128", ones128[:], (1, H), FP16)
            e1drep_sb = load("e1drep", e1drep[:], (L - 1, 16 * H), FP16)

            # d2ones ping-pong (row 32k: d2 strip k; row 32k+1: ones)
            d2o = [consts.tile([66, 16 * NJ], FP16, tag=f"d2o{k}",
                               name=f"d2o{k}")
                   for k in range(2)]
            for t in d2o:
                for k in range(3):
                    nc.sync.dma_start(out=t[32 * k + 1:32 * k + 2, :],
                                      in_=ones6k[:])
            combC = consts.tile([66, 16 * H], FP16, tag="combC")
            combE = consts.tile([66, 16 * H], FP16, tag="combE")

            # ---- phase 0 tail: bias add + single initial h-AG ----
            nc.vector.tensor_tensor(out=h_my[:], in0=h_my, in1=embbT_sb,
                                    op=OP.add)
            hb0 = cp.tile([H, NI], FP16, tag="hagb0")
            nc.vector.tensor_copy(hb0, h_my)
            nc.sync.dma_start(out=hag_in[0][:], in_=hb0)
            nc.gpsimd.collective_compute(
                "AllGather", OP.bypass, replica_groups=rg,
                ins=[hag_in[0][:]], outs=[hag_out[0][:]],
            )

            def send_h(l, hmy_t):
                hb = cp.tile([H, NI], FP16, tag="hagb")
                nc.vector.tensor_copy(hb, hmy_t)
                nc.sync.dma_start(out=hag_in[l][:], in_=hb)
                nc.gpsimd.collective_compute(
                    "AllGather", OP.bypass, replica_groups=rg,
                    ins=[hag_in[l][:]], outs=[hag_out[l][:]],
                )

            def recv_h(l):
                hT = cp.tile([H, NJ], FP16, tag="hT")
                engs = [nc.sync, nc.gpsimd, nc.scalar]
                for r in range(NC):
                    engs[r % 3].dma_start(out=hT[:, r * NI:(r + 1) * NI],
                                          in_=hag_out[l][r * H:(r + 1) * H, :])
                return hT

            def send_h_half(l, half, hmy_t):
                # columns {0-7,16-23,32-39} (half 0) / {8-15,24-31,40-47}
                hv = hmy_t.rearrange("p (b c) -> p b c", b=3)
                hb = cp.tile([H, 3, 8], FP16, tag=f"hagb{half}")
                nc.vector.tensor_copy(hb, hv[:, :, 8 * half:8 * half + 8])
                nc.sync.dma_start(out=hag_h[l][half][:], in_=hb)
                nc.gpsimd.collective_compute(
                    "AllGather", OP.bypass, replica_groups=rg,
                    ins=[hag_h[l][half][:]], outs=[hag_ho[l][half][:]],
                )

            def recv_h_halves(l):
                hT = cp.tile([H, NJ], FP16, tag="hT")
                hv = hT.rearrange("p (rr b c) -> p rr b c", rr=NC, b=3)
                engs = [nc.sync, nc.gpsimd, nc.scalar]
                for half in range(2):
                    for r in range(NC):
                        src_ap = hag_ho[l][half][r * H:(r + 1) * H, :]
                        engs[r % 3].dma_start(
                            out=hv[:, r, :, 8 * half:8 * half + 8],
                            in_=src_ap.rearrange("p (b c) -> p b c", b=3))
                return hT

            x_my = x0my_sb

            # ---- per-layer prep: diff/d2 + fp16 d2 strips ----
            def prep_geom(l, x_t):
                diff = []
                for c in range(D):
                    xb = cp.tile([NI, NJ], F32, tag=f"xb{c}")
                    if l == 0:
                        bsrc = bass.AP(tensor=x0rows, offset=c * NJ,
                                       ap=[[0, NI], [1, NJ]])
                    else:
                        bsrc = bass.AP(tensor=xag_out[l - 1], offset=c * NI,
                                       ap=[[0, NI], [D * NI, NC], [1, NI]])
                    nc.sync.dma_start(out=xb, in_=bsrc)
                    dc = cp.tile([NI, NJ], F32, tag=f"diff{c}")
                    nc.vector.tensor_scalar(
                        out=dc, in0=xb, scalar1=x_t[:, c:c + 1], scalar2=None,
                        op0=OP.subtract,
                    )
                    diff.append(dc)
                d2 = cp.tile([NI, NJ], F32, tag="d2")
                tmp = cp.tile([NI, NJ], F32, tag="ctmp")
                nc.vector.tensor_tensor(out=d2, in0=diff[0], in1=diff[0],
                                        op=OP.mult)
                nc.vector.tensor_tensor(out=tmp, in0=diff[1], in1=diff[1],
                                        op=OP.mult)
                nc.vector.tensor_tensor(out=d2, in0=d2, in1=tmp, op=OP.add)
                nc.vector.tensor_tensor(out=tmp, in0=diff[2], in1=diff[2],
                                        op=OP.mult)
                nc.vector.tensor_tensor(out=d2, in0=d2, in1=tmp, op=OP.add)
                d2s = cp.tile([NI, NJ], F32, tag="d2s")
                nc.vector.tensor_tensor(out=d2s, in0=d2, in1=eyec_sb, op=OP.add)
                d2b = cp.tile([NI, NJ], FP16, tag="d2b")
                nc.vector.tensor_copy(d2b, d2)
                dst = d2o[l % 2]
                for k in range(3):
                    nc.sync.dma_start(out=dst[32 * k:32 * k + 1, :],
                                      in_=d2b[16 * k:16 * (k + 1), :])
                # u = 1/(1+sqrt(d2s)) off the layer-end critical path
                dn = cp.tile([NI, NJ], F32, tag="dn")
                nc.scalar.activation(out=dn, in_=d2s, func=AF.Sqrt)
                nc.vector.tensor_scalar(out=dn, in0=dn, scalar1=1.0,
                                        scalar2=None, op0=OP.add)
                u = cp.tile([NI, NJ], F32, tag="u")
                nc.vector.reciprocal(out=u, in_=dn)
                return diff, u

            def make_comb(l, hiT_l, wdrep_l, comb, hmy_t, atag):
                At_ps = ps_acc.tile([NI, H], F32, tag="acc")
                nc.tensor.matmul(At_ps, lhsT=hmy_t, rhs=hiT_l, start=True,
                                 stop=True)
                At = cp.tile([NI, H], FP16, tag=atag)
                nc.vector.tensor_copy(At, At_ps)
                for k in range(3):
                    nc.sync.dma_start(out=comb[32 * k:32 * k + 1, :],
                                      in_=wdrep_l)
                    nc.sync.dma_start(out=comb[32 * k + 1:32 * k + 2, :],
                                      in_=At[16 * k:16 * (k + 1), :])

            diff, u_t = prep_geom(0, x0my_sb)
            hT_bf = recv_h(0)
            make_comb(0, c1hiT_sb[:, 0, :], c1drep_sb[0:1, :], combC, h_my,
                      "AtC")
            make_comb(0, e1hiT_sb[:, 0, :], e1drep_sb[0:1, :], combE, h_my,
                      "AtE")

            for l in range(L):
                last = l == L - 1
                d2l = d2o[l % 2]
                hT_l = hT_bf
                hmy_l = h_my

                # one MLP group g covers rows (g, g+16, g+32): the three K=2
                # distance matmuls hit PE row-blocks 0/32/64 and run packed.
                def emit_group(comb, b1c, hjT, w2T, b2c, g, slab):
                    pre = ps_mlp.tile([H, G * 512], F32, tag="mlp")
                    for r in range(G):
                        nc.tensor.matmul(
                            pre[:, r * 512:r * 512 + NJ],
                            lhsT=comb[32 * r:32 * r + 2,
                                      g * H:(g + 1) * H],
                            rhs=d2l[32 * r:32 * r + 2,
                                    g * NJ:(g + 1) * NJ],
                            start=True, stop=False)
                    for r in range(G):
                        nc.tensor.matmul(
                            pre[:, r * 512:r * 512 + NJ],
                            lhsT=hjT, rhs=hT_l,
                            start=False, stop=True)
                    t1 = work.tile([H, G * NJ], FP16, tag="t1", bufs=3)
                    nc.scalar.activation(
                        out=t1[:, :].rearrange("p (r c) -> p r c", r=G),
                        in_=pre.rearrange("p (r c) -> p r c", r=G)[:, :, 0:NJ],
                        func=AF.Silu, bias=b1c, scale=1.0,
                    )
                    z2 = pre  # reuse the pre banks: t1 has been read out
                    nc.tensor.matmul(z2[:, 0:512], lhsT=w2T, rhs=t1[:, 0:512],
                                     start=True, stop=True)
                    nc.tensor.matmul(z2[:, 512:1024], lhsT=w2T,
                                     rhs=t1[:, 512:1024], start=True, stop=True)
                    nc.tensor.matmul(z2[:, 1024:1152], lhsT=w2T,
                                     rhs=t1[:, 1024:1152], start=True,
                                     stop=True)
                    if slab is not None:
                        t2 = slab[:, g * (G * NJ):(g + 1) * (G * NJ)]
                    else:
                        t2 = work.tile([H, G * NJ], FP16, tag="t2", bufs=3)
                    nc.scalar.activation(
                        out=t2, in_=z2[:, 0:G * NJ],
                        func=AF.Silu, bias=b2c, scale=1.0,
                    )
                    return t2

                phi_ps = ps_acc.tile([H, NJ], F32, tag="acc")
                pendC = []

                def flush_coord(item):
                    t2p, gp = item
                    for r in range(G):
                        i = _row(gp, r)
                        nc.tensor.matmul(
                            phi_ps[0:NI, :],
                            lhsT=c3w_sb[:, l, (NI - 1) - i:(2 * NI - 1) - i],
                            rhs=t2p[:, r * NJ:(r + 1) * NJ],
                            start=(gp == 0 and r == 0),
                            stop=(gp == NGRP - 1 and r == G - 1),
                        )

                xnew = cp.tile([NI, D], F32, tag="xnew")

                def x_chain():
                    u = u_t
                    phis = cp.tile([NI, NJ], F32, tag="phis")
                    nc.vector.tensor_scalar(out=phis, in0=phi_ps[0:NI, :],
                                            scalar1=cb3c_sb[:, l:l + 1],
                                            scalar2=None, op0=OP.add)
                    s = cp.tile([NI, NJ], F32, tag="s")
                    nc.vector.tensor_tensor(out=s, in0=phis, in1=u, op=OP.mult)
                    nc.vector.tensor_tensor(out=s, in0=s, in1=maskc_sb,
                                            op=OP.mult)
                    for c in range(D):
                        xm = mgp.tile([NI, NJ], F32, tag="xm")
                        xcol = cp.tile([NI, 1], F32, tag=f"xcol{c}")
                        nc.vector.scalar_tensor_tensor(
                            out=xm, in0=diff[c], scalar=1.0, in1=s,
                            op0=OP.mult, op1=OP.mult, accum_out=xcol)
                        nc.vector.tensor_tensor(out=xnew[:, c:c + 1], in0=xcol,
                                                in1=x_my[:, c:c + 1],
                                                op=OP.add)
                    if not last:
                        nc.sync.dma_start(out=xag_in[l]
                                          .rearrange("c n -> n c"), in_=xnew)
                    else:
                        nc.sync.dma_start(out=xag_in[l][:], in_=xnew)
                    nc.gpsimd.collective_compute(
                        "AllGather", OP.bypass, replica_groups=rg,
                        ins=[xag_in[l][:]], outs=[xag_out[l][:]],
                    )

                if last:
                    for g in range(NGRP):
                        t2c = emit_group(combC, cb1_sb[:, l:l + 1],
                                         c1hjT_sb[:, l, :], c2T_sb[:, l, :],
                                         cb2_sb[:, l:l + 1], g, None)
                        pendC.append((t2c, g))
                        if len(pendC) > 1:
                            flush_coord(pendC.pop(0))
                    while pendC:
                        flush_coord(pendC.pop(0))
                    x_chain()
                    nc.sync.dma_start(out=o_x[:], in_=xag_out[l][:])
                    break

                # ---------- merged coord+edge phase ----------
                m2slab = slabp.tile([H, NI * NJ], FP16, tag="m2")
                msumT = cp.tile([H, NI], F32, tag="msumT")
                att_sub = [None] * NSUB
                gmb = [None] * NSUB
                pendE = []
                subs_ready = [0]
                subs_done = [0]

                def sigma_sub(k):
                    tt = cp.tile([SUB, NJ], F32, tag="tt")
                    nc.scalar.activation(out=tt, in_=att_sub[k][0:SUB, :],
                                         func=AF.Tanh, scale=0.5)
                    gm = cp.tile([SUB, NJ], FP16, tag="gmb")
                    nc.vector.scalar_tensor_tensor(
                        out=gm, in0=tt, scalar=1.0,
                        in1=hm4_sb[:, k * NJ:(k + 1) * NJ],
                        op0=OP.add, op1=OP.mult)
                    gmb[k] = gm
                    subs_ready[0] += 1

                def flush_edge(gp):
                    t2p = m2slab[:, gp * (G * NJ):(gp + 1) * (G * NJ)]
                    k = gp // 4
                    for r in range(G):
                        slot = (gp % 4) + 4 * r
                        if gp % 4 == 0 and r == 0:
                            att_sub[k] = ps_acc.tile([SUB, NJ], F32,
                                                     tag="acc",
                                                     name=f"attsub{k}")
                        nc.tensor.matmul(
                            att_sub[k][:, :],
                            lhsT=attw_sb[:, l, (SUB - 1) - slot:
                                         (2 * SUB - 1) - slot],
                            rhs=t2p[:, r * NJ:(r + 1) * NJ],
                            start=(gp % 4 == 0 and r == 0),
                            stop=(gp % 4 == 3 and r == G - 1),
                        )
                    if gp % 4 == 3:
                        sigma_sub(k)

                def msum_sub(k):
                    # collapse the 12 gate rows to one partition, then
                    # broadcast each across partitions via a K=1 matmul
                    growc = mgp.tile([1, SUB * NJ], FP16, tag="growc",
                                     bufs=2)
                    nc.sync.dma_start(out=growc, in_=gmb[k])
                    for s in range(SUB):
                        i = _slot_to_row(k, s)
                        off = (i % 16) * (G * NJ) + (i // 16) * NJ
                        gb = ps_acc.tile([H, NJ], F32, tag="acc", name="gb")
                        nc.tensor.matmul(
                            gb, lhsT=ones_sb,
                            rhs=growc[0:1, s * NJ:(s + 1) * NJ],
                            start=True, stop=True)
                        mg = mgp.tile([H, NJ], FP16, tag="mg")
                        nc.vector.scalar_tensor_tensor(
                            out=mg, in0=m2slab[:, off:off + NJ],
                            scalar=1.0, in1=gb,
                            op0=OP.mult, op1=OP.mult,
                            accum_out=msumT[:, i:i + 1])

                h_my_new = cp.tile([H, NI], F32, tag="hmy")

                def node_half(half):
                    # node MLP on 24 local nodes: cols {0-7,16-23,32-39}+8*half
                    sl = slice(8 * half, 8 * half + 8)
                    hv = hmy_l.rearrange("p (b c) -> p b c", b=3)
                    mv = msumT.rearrange("p (b c) -> p b c", b=3)
                    z1m = ps_acc.tile([H, 24], F32, tag="acc",
                                      name=f"z1m{half}")
                    nc.tensor.matmul(z1m, lhsT=nw1hT_sb[:, l, :],
                                     rhs=hv[:, :, sl], start=True, stop=False)
                    nc.tensor.matmul(z1m, lhsT=nw1mT_sb[:, l, :],
                                     rhs=mv[:, :, sl], start=False, stop=True)
                    t1nm = cp.tile([H, 24], F32, tag=f"t1nm{half}")
                    nc.scalar.activation(out=t1nm, in_=z1m, func=AF.Silu,
                                         bias=nb1_sb[:, l:l + 1], scale=1.0)
                    z2m = ps_acc.tile([H, 24], F32, tag="acc",
                                      name=f"z2m{half}")
                    nc.tensor.matmul(z2m, lhsT=nw2T_sb[:, l, :], rhs=t1nm,
                                     start=True, stop=True)
                    nv = h_my_new.rearrange("p (b c) -> p b c", b=3)
                    nc.vector.tensor_scalar(out=nv[:, :, sl], in0=z2m,
                                            scalar1=nb2_sb[:, l:l + 1],
                                            scalar2=None, op0=OP.add)
                    send_h_half(l + 1, half, h_my_new)

                # slot schedule: 2 coord : 1 edge until coords exhausted,
                # then the remaining edge groups
                slots = []
                for q in range(8):
                    slots += ["C", "C", "E"]
                slots += ["E"] * 8
                ci = [0]
                ei = [0]
                xdone = [False]
                prep_at = [None]

                for si, kind in enumerate(slots):
                    if kind == "C":
                        t2c = emit_group(combC, cb1_sb[:, l:l + 1],
                                         c1hjT_sb[:, l, :], c2T_sb[:, l, :],
                                         cb2_sb[:, l:l + 1], ci[0], None)
                        pendC.append((t2c, ci[0]))
                        ci[0] += 1
                        if len(pendC) > 1:
                            flush_coord(pendC.pop(0))
                    else:
                        emit_group(combE, eb1_sb[:, l:l + 1],
                                   e1hjT_sb[:, l, :], e2T_sb[:, l, :],
                                   eb2_sb[:, l:l + 1], ei[0], m2slab)
                        pendE.append(ei[0])
                        ei[0] += 1
                        if len(pendE) > 1:
                            flush_edge(pendE.pop(0))
                    if ci[0] == NGRP and not xdone[0]:
                        while pendC:
                            flush_coord(pendC.pop(0))
                        x_chain()
                        xdone[0] = True
                        prep_at[0] = si + 3
                    if prep_at[0] is not None and si == prep_at[0]:
                        diff, u_t = prep_geom(l + 1, xnew)
                    while xdone[0] and subs_done[0] < subs_ready[0]:
                        msum_sub(subs_done[0])
                        subs_done[0] += 1
                        if subs_done[0] == 2:
                            node_half(0)
                while pendE:
                    flush_edge(pendE.pop(0))
                while subs_done[0] < subs_ready[0]:
                    msum_sub(subs_done[0])
                    subs_done[0] += 1
                    if subs_done[0] == 2:
                        node_half(0)
                node_half(1)
                h_my = h_my_new

                # prep for next layer while the h AllGather flies
                make_comb(l + 1, c1hiT_sb[:, l + 1, :],
                          c1drep_sb[l + 1:l + 2, :], combC, h_my, "AtC")
                if l + 1 < L - 1:
                    make_comb(l + 1, e1hiT_sb[:, l + 1, :],
                              e1drep_sb[l + 1:l + 2, :], combE, h_my, "AtE")
                x_my = xnew
                hT_bf = recv_h_halves(l + 1)

    nc.finalize()
    return nc


def _prep_inputs(inputs):
    """Host-side prep: per-core input maps from full arrays."""
    F16 = np.float16
    f = lambda a: np.ascontiguousarray(np.asarray(a), dtype=np.float32)
    b = lambda a: np.ascontiguousarray(np.asarray(a, dtype=np.float32)
                                       .astype(F16))
    x_inp = f(inputs["x_inp"])
    emb_w = f(inputs["emb_w"])
    emb_b = f(inputs["emb_b"])
    coord_w1 = f(inputs["coord_w1"])
    coord_b1 = f(inputs["coord_b1"])
    coord_w2 = f(inputs["coord_w2"])
    coord_b2 = f(inputs["coord_b2"])
    coord_w3 = f(inputs["coord_w3"])
    coord_b3 = f(inputs["coord_b3"])
    edge_w1 = f(inputs["edge_w1"])
    edge_b1 = f(inputs["edge_b1"])
    edge_w2 = f(inputs["edge_w2"])
    edge_b2 = f(inputs["edge_b2"])
    node_w1 = f(inputs["node_w1"])
    node_b1 = f(inputs["node_b1"])
    node_w2 = f(inputs["node_w2"])
    node_b2 = f(inputs["node_b2"])
    att_w = f(inputs["att_w"])

    x0 = x_inp.reshape(N, D)
    eye = np.eye(N, dtype=np.float32)

    def stackT(w, lo, hi):
        return np.ascontiguousarray(
            np.stack([w[l, :, lo:hi].T for l in range(w.shape[0])]))

    def win(w3, S):
        nl = w3.shape[0]
        out = np.zeros((nl, H, 2 * S - 1), np.float32)
        out[:, :, S - 1] = w3[:, 0, :]
        return out

    shared = dict(
        x0rows=np.ascontiguousarray(x0.T.reshape(1, D * N)),
        c1hiT=stackT(coord_w1, 0, H),
        c1hjT=b(stackT(coord_w1, H, 2 * H)),
        c1drep=b(np.tile(coord_w1[:, :, 2 * H], (1, 16))),
        cb1=np.ascontiguousarray(coord_b1.T),
        c2T=b(np.stack([coord_w2[l].T for l in range(L)])),
        cb2=np.ascontiguousarray(coord_b2.T),
        c3w=b(win(coord_w3, NI)),
        cb3c=np.ascontiguousarray(
            np.broadcast_to(coord_b3[:, 0][None, :], (NI, L))),
        e1hiT=stackT(edge_w1, 0, H),
        e1hjT=b(stackT(edge_w1, H, 2 * H)),
        e1drep=b(np.tile(edge_w1[:, :, 2 * H], (1, 16))),
        ones6k=b(np.ones((1, 16 * NJ), np.float32)),
        eb1=np.ascontiguousarray(edge_b1.T),
        e2T=b(np.stack([edge_w2[l].T for l in range(L - 1)])),
        eb2=np.ascontiguousarray(edge_b2.T),
        attw=b(win(att_w, SUB)),
        nw1hT=stackT(node_w1, 0, H),
        nw1mT=stackT(node_w1, H, 2 * H),
        nb1=np.ascontiguousarray(node_b1.T),
        nw2T=np.ascontiguousarray(np.stack([node_w2[l].T
                                            for l in range(L - 1)])),
        nb2=np.ascontiguousarray(node_b2.T),
        ones128=b(np.ones((1, H), np.float32)),
    )
    in_maps = []
    for c in range(NC):
        m = dict(shared)
        m["embw"] = np.ascontiguousarray(
            emb_w[c * EMB_ROWS:(c + 1) * EMB_ROWS, :])
        m["embbT"] = np.ascontiguousarray(
            emb_b[c * EMB_ROWS:(c + 1) * EMB_ROWS].reshape(NI, H).T)
        m["x0my"] = np.ascontiguousarray(x0[c * NI:(c + 1) * NI, :])
        mk = 1.0 - eye[c * NI:(c + 1) * NI, :]
        m["maskc"] = np.ascontiguousarray(mk)
        m["eyec"] = np.ascontiguousarray(eye[c * NI:(c + 1) * NI, :])
        # 0.5*mask rows in (sub, slot) layout
        hm = np.zeros((SUB, NSUB * NJ), np.float32)
        for k in range(NSUB):
            for s in range(SUB):
                hm[s, k * NJ:(k + 1) * NJ] = 0.5 * mk[_slot_to_row(k, s), :]
        m["hm4"] = np.ascontiguousarray(hm)
        in_maps.append(m)
    return in_maps


def _run(inputs, trace=False, **kw):
    from concourse.bass_utils import run_bass_kernel_spmd
    if "nc" not in _cache:
        _cache["nc"] = _build_nc()
    in_maps = _prep_inputs(inputs)
    return run_bass_kernel_spmd(_cache["nc"], in_maps, list(range(NC)),
                                trace=trace, **kw)


def kernel(**inputs) -> np.ndarray:
    res = _run(inputs)
    return np.asarray(res.results[0]["o_x"], dtype=np.float32).reshape(N * D)
